# revision 1
# baseline (speedup 1.0000x reference)
"""Trainium2 Bass kernel for AffinityNeuralNetworkMONN (gnn_message_passing).

Sharding: data-parallel over B=128 graphs -> 8 NeuronCores x 16 graphs.
Inside a core, graphs are processed in waves; per-graph heavy tensors use
a [H=128 partitions, nodes free] (T) layout so ACT bias/scale fuse per
partition; node-contraction operands are built NM via PE transposes.
All matmuls run as float32r (full-rate at N>=256). Softmax score rows are
reduced across partitions on GPSIMD and scattered to node-major via DMA.
Only tanh/exp/prelu ACT functions are used (one table set, no reloads).
"""
import sys
for p in ("/opt/trn_rl_repo", "/root/.axon_site/_ro/trn_rl_repo"):
    if p not in sys.path:
        sys.path.insert(0, p)

import numpy as np
import os
from contextlib import ExitStack

import concourse.bass as bass
import concourse.tile as tile
from concourse import mybir, masks
from concourse import bass_isa
from concourse.bass_utils import run_bass_kernel_spmd

F32 = mybir.dt.float32
F32R = mybir.dt.float32r
A = mybir.ActivationFunctionType
OP = mybir.AluOpType
AX = mybir.AxisListType

NCORES = 8
B, NC, NP, H, D = 128, 64, 1024, 128, 3
G = B // NCORES            # graphs per core = 16
WAVES = [2] * 8            # wave sizes (sum = 16); wv<=2 verified on HW
if os.environ.get("KWAVES"):
    WAVES = [int(x) for x in os.environ["KWAVES"].split(",")]
NCHUNK = NP // 128         # 8 p-chunks per graph

W_PC, W_PP, W_CAFF, W_PAFF, W_SAFF = 0, 128, 256, 384, 512
def W_C2P(i): return 640 + i * 128
def W_HC0(i): return 1024 + i * 128
def W_P2C(i): return 1408 + i * 128
def W_HP0(i): return 1792 + i * 128
def W_MCP(i): return 2176 + i * 256
W_IH, W_HH = 2944, 3328
def W_HC1(i): return 3712 + i
def W_HP1(i): return 3715 + i
WCOLS = 3718

B_PP, B_PAFF, B_PC, B_CAFF = 0, 1, 2, 3
def B_C2P(i): return 4 + i
def B_HC0(i): return 7 + i
def B_P2C(i): return 10 + i
def B_HP0(i): return 13 + i
BCOLS = 16

BG_SAFF = 0
def BG_MCP(i): return 128 + i * 256
BG_IH, BG_HH = 896, 1280
BGCOLS = 1664

_CACHE = {}
TRACE = False
LAST_EXEC_NS = None


def _split_waits(nc, keep=1):
    """walrus allows very few attached sync-waits per instruction (1 for the
    f32 self-loading matmul struct). Hoist excess waits into standalone
    EventSemaphore instructions right before the over-subscribed one."""
    for fn in nc.m.functions:
        for blk in fn.blocks:
            out = []
            for ins in blk.instructions:
                si = ins.sync_info
                if si is not None and si.on_wait and len(si.on_wait) > keep:
                    waits = list(si.on_wait)
                    for jj, w in enumerate(waits[:-keep]):
                        ev = mybir.InstNoOp(
                            name=f"{ins.name}-wsplit{jj}",
                            sync_info=mybir.SyncInfo(on_wait=[w], on_update=[]),
                            bass_nofuse=True)
                        ev.engine = ins.engine
                        out.append(ev)
                    si.on_wait = waits[-keep:]
                    ins.sync_info = si
                out.append(ins)
            blk.instructions = out


def _build(b_out_val: float, split: bool = True, sim_compat: bool = False):
    bisect = os.environ.get("KBISECT", "0") == "1"
    nc = bass.Bass()
    protT_d = nc.dram_tensor("protT", [H, G * NP], F32, kind="ExternalInput")
    compT_d = nc.dram_tensor("compT", [H, G * NC], F32, kind="ExternalInput")
    gompT_d = nc.dram_tensor("gompT", [H, G], F32, kind="ExternalInput")
    wpack_d = nc.dram_tensor("wpack", [H, WCOLS], F32, kind="ExternalInput")
    bpack_d = nc.dram_tensor("bpack", [H, BCOLS], F32, kind="ExternalInput")
    bg16_d = nc.dram_tensor("bg16", [G, BGCOLS], F32, kind="ExternalInput")
    w2t_d = nc.dram_tensor("w2t", [H, 2 * H], F32, kind="ExternalInput")
    out_d = nc.dram_tensor("out", [G, 1], F32, kind="ExternalOutput")

    with tile.TileContext(nc) as tc, ExitStack() as ctx:
        gl_pool = ctx.enter_context(tc.tile_pool(name="globals", bufs=1))
        per_pool = ctx.enter_context(tc.tile_pool(name="persist", bufs=1))
        st_pool = ctx.enter_context(tc.tile_pool(name="stream", bufs=2))
        sm_pool = ctx.enter_context(tc.tile_pool(name="small", bufs=2))
        ps_big = ctx.enter_context(tc.tile_pool(name="psBig", bufs=2, space="PSUM"))
        ps_t = ctx.enter_context(tc.tile_pool(name="psT", bufs=2, space="PSUM"))
        ps_g = ctx.enter_context(tc.tile_pool(name="psG", bufs=1, space="PSUM"))
        dr_pool = ctx.enter_context(tc.tile_pool(name="dram", bufs=2, space="DRAM"))

        def psg():
            return ps_g.tile([H, 512], F32, name="psg", tag="psg")

        def psg2():
            return ps_g.tile([H, 512], F32, name="psg2", tag="psg2")

        def act_lrelu(dst, src_ps, bias=0.0, accum_out=None):
            if not sim_compat:
                nc.scalar.activation(dst, src_ps, A.Prelu, bias=bias, alpha=0.1,
                                     accum_out=accum_out)
            else:
                shp = [dst.shape[0], int(np.prod(dst.shape[1:]))]
                t1 = sm_pool.tile(shp, F32, name="lr1", tag="lr_t1", bufs=1)
                t2 = sm_pool.tile(shp, F32, name="lr2", tag="lr_t2", bufs=1)
                nb = bias if isinstance(bias, float) else None
                nc.scalar.activation(t1[:], src_ps, A.Relu, bias=bias)
                if nb is None:
                    negb = sm_pool.tile([dst.shape[0], 1], F32, name="lrnb",
                                        tag="lr_nb", bufs=1)
                    nc.vector.tensor_scalar(negb[:], bias, -1.0, None, OP.mult)
                    nc.scalar.activation(t2[:], src_ps, A.Relu, scale=-1.0, bias=negb[:])
                else:
                    nc.scalar.activation(t2[:], src_ps, A.Relu, scale=-1.0, bias=-nb)
                nc.vector.scalar_tensor_tensor(dst, t2[:], -0.1, t1[:],
                                               OP.mult, OP.add, accum_out=accum_out)

        # ---------- preamble ----------
        wp = gl_pool.tile([H, WCOLS], F32R, name="wp", tag="wp")
        bp = gl_pool.tile([H, BCOLS], F32, name="bp", tag="bp")
        bg = gl_pool.tile([G, BGCOLS], F32, name="bg", tag="bg")
        w2t = gl_pool.tile([H, 2 * H], F32, name="w2t", tag="w2t")
        compT = gl_pool.tile([H, G * NC], F32R, name="compT", tag="compT")
        gompT = gl_pool.tile([H, G], F32R, name="gompT", tag="gompT")
        nc.sync.dma_start(out=wp[:], in_=wpack_d[:].bitcast(F32R))
        nc.sync.dma_start(out=bp[:], in_=bpack_d[:])
        nc.sync.dma_start(out=bg[:], in_=bg16_d[:])
        nc.sync.dma_start(out=w2t[:], in_=w2t_d[:])
        nc.sync.dma_start(out=compT[:], in_=compT_d[:].bitcast(F32R))
        nc.sync.dma_start(out=gompT[:], in_=gompT_d[:].bitcast(F32R))

        ident = gl_pool.tile([H, H], F32, name="ident", tag="ident")
        masks.make_identity(nc, ident[:])
        ones_r = gl_pool.tile([H, 2], F32R, name="ones_r", tag="ones_r")
        nc.vector.memset(ones_r[:].bitcast(F32), 1.0)
        identr = ident[:].bitcast(F32R)

        ceT = gl_pool.tile([H, G * NC], F32R, name="ceT", tag="ceT")
        pcT = gl_pool.tile([H, G * NC], F32R, name="pcT", tag="pcT")
        for (dst, wcol, bcol) in ((ceT, W_CAFF, B_CAFF), (pcT, W_PC, B_PC)):
            pscc = ps_big.tile([H, G * NC], F32, name="big", tag="big")
            nc.tensor.matmul(pscc[:, 0:512], wp[:, wcol:wcol + H], compT[:, 0:512],
                             start=True, stop=True)
            nc.tensor.matmul(pscc[:, 512:1024], wp[:, wcol:wcol + H], compT[:, 512:1024],
                             start=True, stop=True)
            act_lrelu(dst[:], pscc[:], bias=bp[:, bcol:bcol + 1])

        # CE_NM [128, 8, 128]: pair-transposed ce (abs graphs 2k, 2k+1 stacked)
        ce_nm = gl_pool.tile([H, 8, H], F32R, name="ce_nm", tag="ce_nm")
        for half in range(2):
            pst = ps_t.tile([H, 512], F32, name="pst", tag="pst")
            for k in range(4):
                pr = half * 4 + k
                nc.tensor.transpose(pst[:, k * 128:(k + 1) * 128],
                                    ceT[:, pr * 128:(pr + 1) * 128].bitcast(F32), ident[:])
            nc.vector.tensor_copy(ce_nm[:, half * 4:(half + 1) * 4, :],
                                  pst[:].rearrange("h (k c) -> h k c", k=4))

        cesum = gl_pool.tile([H, G], F32, name="cesum", tag="cesum")
        nc.vector.tensor_reduce(cesum[:],
                                ceT[:].bitcast(F32).rearrange("h (g c) -> h g c", g=G),
                                AX.X, OP.add)
        peacc = gl_pool.tile([H, G], F32, name="peacc", tag="peacc")
        partials = gl_pool.tile([H, G], F32R, name="partials", tag="partials")

        # ---------- waves ----------
        g0 = 0
        for wv in WAVES:
            gs, ge = g0, g0 + wv
            g0 = ge

            peT = [per_pool.tile([H, NP], F32R, name=f"peT{j}", tag=f"peT{j}") for j in range(wv)]
            pairslab = [per_pool.tile([H, NP], F32R, name=f"pairs{q}", tag=f"pairs{q}")
                        for q in range((wv + 1) // 2)]
            pair = [pairslab[j // 2][(j % 2) * 64:(j % 2) * 64 + 64, :] for j in range(wv)]
            pwe = [per_pool.tile([H, NCHUNK, 65], F32R, name=f"pwe{j}", tag=f"pwe{j}") for j in range(wv)]
            ppe = [per_pool.tile([H, NCHUNK, 256], F32R, name=f"ppe{j}", tag=f"ppe{j}") for j in range(wv)]

            # ----- phase A -----
            for j in range(wv):
                g = gs + j
                protT = st_pool.tile([H, NP], F32R, name="protT", tag="protT")
                nc.sync.dma_start(out=protT[:],
                                  in_=protT_d[:, g * NP:(g + 1) * NP].bitcast(F32R))

                ps_pp = ps_big.tile([H, NP], F32, name="big", tag="big")
                nc.tensor.matmul(ps_pp[:, 0:512], wp[:, W_PP:W_PP + H], protT[:, 0:512],
                                 start=True, stop=True)
                nc.tensor.matmul(ps_pp[:, 512:1024], wp[:, W_PP:W_PP + H],
                                 protT[:, 512:1024], start=True, stop=True)
                ppT = st_pool.tile([H, NP], F32R, name="ppT", tag="ppT")
                act_lrelu(ppT[:], ps_pp[:], bias=bp[:, B_PP:B_PP + 1])

                ps_pe = ps_big.tile([H, NP], F32, name="big", tag="big")
                nc.tensor.matmul(ps_pe[:, 0:512], wp[:, W_PAFF:W_PAFF + H], protT[:, 0:512],
                                 start=True, stop=True)
                nc.tensor.matmul(ps_pe[:, 512:1024], wp[:, W_PAFF:W_PAFF + H],
                                 protT[:, 512:1024], start=True, stop=True)
                act_lrelu(peT[j][:], ps_pe[:], bias=bp[:, B_PAFF:B_PAFF + 1],
                          accum_out=peacc[:, g:g + 1])

                # pairwise = sigmoid(pc @ pp^T) = 0.5 + 0.5*tanh(z/2)
                hb = (j % 2) * 64
                ps_pw = ps_big.tile([H, NP], F32, name="big", tag="big")
                nc.tensor.matmul(ps_pw[0:64, 0:512], pcT[:, g * NC:(g + 1) * NC],
                                 ppT[:, 0:512], start=True, stop=True)
                nc.tensor.matmul(ps_pw[0:64, 512:1024], pcT[:, g * NC:(g + 1) * NC],
                                 ppT[:, 512:1024], start=True, stop=True)
                pw_t = st_pool.tile([H, NP], F32, name="pw_t", tag="pw_t")
                nc.scalar.activation(pw_t[0:64, :], ps_pw[0:64, :], A.Tanh, scale=0.5)
                if hb == 0:
                    nc.vector.tensor_scalar(pair[j], pw_t[0:64, :], 0.5, 0.5,
                                            OP.mult, OP.add)
                else:
                    pair_st = st_pool.tile([64, NP], F32, name="pair_st", tag="pair_st")
                    nc.vector.tensor_scalar(pair_st[:], pw_t[0:64, :], 0.5, 0.5,
                                            OP.mult, OP.add)
                    nc.sync.dma_start(out=pair[j].bitcast(F32), in_=pair_st[:])

                # pairwiseT -> pwe cols 0:64
                for half in range(2):
                    pstp = ps_t.tile([H, 512], F32, name="pst", tag="pst")
                    for k in range(4):
                        ch = half * 4 + k
                        nc.tensor.transpose(pstp[:, k * 128:k * 128 + 64],
                                            pair[j][:, ch * 128:(ch + 1) * 128].bitcast(F32),
                                            ident[hb:hb + 64, hb:hb + 64])
                    nc.vector.tensor_copy(
                        pwe[j][:, half * 4:(half + 1) * 4, 0:64],
                        pstp[:].rearrange("h (k c) -> h k c", k=4)[:, :, 0:64])

                # peT transposes -> ppe cols 128:256 (pe_NM)
                for half in range(2):
                    psq = ps_t.tile([H, 512], F32, name="pst", tag="pst")
                    for k in range(4):
                        ch = half * 4 + k
                        nc.tensor.transpose(psq[:, k * 128:(k + 1) * 128],
                                            peT[j][:, ch * 128:(ch + 1) * 128].bitcast(F32),
                                            ident[:])
                    nc.vector.tensor_copy(
                        ppe[j][:, half * 4:(half + 1) * 4, 128:256],
                        psq[:].rearrange("h (k c) -> h k c", k=4))

            # sf for this wave: lrelu(gomp @ W_saff + b_saff)
            ps_sf = psg()
            nc.tensor.matmul(ps_sf[0:wv, 0:256], gompT[:, gs:ge],
                             wp[:, W_SAFF:W_SAFF + 256], start=True, stop=True)
            sf_pre = sm_pool.tile([wv, H], F32, name="sf_pre", tag="sf_pre", bufs=1)
            nc.vector.tensor_add(sf_pre[:], ps_sf[0:wv, 0:H], bg[0:wv, BG_SAFF:BG_SAFF + H])
            sf_w = sm_pool.tile([wv, H], F32, name="sf_w", tag="sf_w")
            act_lrelu(sf_w[:], sf_pre[:])

            # m0
            mT = sm_pool.tile([H, wv], F32R, name="mT", tag="mT")
            nc.vector.scalar_tensor_tensor(mT[:], cesum[:, gs:ge], 1.0 / (NC * NP),
                                           peacc[:, gs:ge], OP.mult, OP.mult)
            ps_m0 = psg()
            nc.tensor.transpose(ps_m0[0:wv, 0:H], mT[:].bitcast(F32), ident[:])
            m_nm = sm_pool.tile([wv, H], F32, name="m_nm", tag="m_nm")
            nc.vector.tensor_copy(m_nm[:], ps_m0[0:wv, 0:H])

            xcf = sm_pool.tile([wv, H], F32, name="xcf", tag="xcf")
            pfn = sm_pool.tile([wv, H], F32R, name="pfn", tag="pfn")

            # ----- phase B: D iterations -----
            for i in range(D):
                csl = slice(gs * NC, ge * NC)
                ps_cp = ps_big.tile([H, wv * NC], F32, name="big", tag="big")
                nc.tensor.matmul(ps_cp[:], wp[:, W_C2P(i):W_C2P(i) + H], ceT[:, csl],
                                 start=True, stop=True)
                cpreT = sm_pool.tile([H, wv * NC], F32, name="cpreT", tag="cpreT")
                nc.scalar.activation(cpreT[:], ps_cp[:], A.Tanh,
                                     bias=bp[:, B_C2P(i):B_C2P(i) + 1])
                ps_h0 = ps_big.tile([H, wv * NC], F32, name="big", tag="big")
                nc.tensor.matmul(ps_h0[:], wp[:, W_HC0(i):W_HC0(i) + H], ceT[:, csl],
                                 start=True, stop=True)
                hc0T = sm_pool.tile([H, wv * NC], F32, name="hc0T", tag="hc0T")
                nc.scalar.activation(hc0T[:], ps_h0[:], A.Tanh,
                                     bias=bp[:, B_HC0(i):B_HC0(i) + 1])

                # c_pre_NM (graph-pair transposes; wave starts are odd-free:
                # waves [5,5,3] start at 0,5,10 -> pairs may straddle; use
                # per-graph 64-col transposes into fixed parity slots)
                cpre_nm = sm_pool.tile([H, 2, H], F32R, name="cpre_nm", tag="cpre_nm")
                psq2 = ps_t.tile([H, 512], F32, name="pst", tag="pst")
                for j in range(wv):
                    nc.tensor.transpose(psq2[0:64, j * 128:(j + 1) * 128],
                                        cpreT[:, j * 64:(j + 1) * 64], ident[:])
                # evens -> partitions 0:64 (DVE), odds -> 64:128 (DMA shifts partitions)
                ne, no = (wv + 1) // 2, wv // 2
                psq2v = psq2[0:64, 0:wv * 128].rearrange("c (j h) -> c j h", j=wv)
                nc.vector.tensor_copy(cpre_nm[0:64, 0:ne, :], psq2v[:, 0::2, :])
                cpre_odd = sm_pool.tile([64, 2, H], F32, name="cpre_odd", tag="cpre_odd", bufs=1)
                nc.vector.tensor_copy(cpre_odd[:, 0:no, :], psq2v[:, 1::2, :])
                nc.sync.dma_start(out=cpre_nm[64:128, 0:no, :].bitcast(F32),
                                  in_=cpre_odd[:, 0:no, :])

                # mc1/mp1 batched
                ps_mm = psg()
                nc.tensor.matmul(ps_mm[0:wv, 0:256], mT[:], wp[:, W_MCP(i):W_MCP(i) + 256],
                                 start=True, stop=True)
                mcp_pre = sm_pool.tile([wv, 256], F32, name="mcp_pre", tag="mcp_pre", bufs=1)
                nc.vector.tensor_add(mcp_pre[:], ps_mm[0:wv, 0:256],
                                     bg[0:wv, BG_MCP(i):BG_MCP(i) + 256])
                mcp = sm_pool.tile([wv, 256], F32, name="mcp", tag="mcp")
                nc.scalar.activation(mcp[:], mcp_pre[:], A.Tanh)
                ps_mt = psg()
                nc.tensor.transpose(ps_mt[0:H, 0:wv], mcp[:, 0:H], ident[0:wv, 0:wv])
                mc1T = sm_pool.tile([H, wv], F32, name="mc1T", tag="mc1T")
                nc.vector.tensor_copy(mc1T[:], ps_mt[0:H, 0:wv])
                ps_mt2 = psg()
                nc.tensor.transpose(ps_mt2[0:H, 0:wv], mcp[:, H:256], ident[0:wv, 0:wv])
                mp1T = sm_pool.tile([H, wv], F32, name="mp1T", tag="mp1T")
                nc.vector.tensor_copy(mp1T[:], ps_mt2[0:H, 0:wv])

                wc_w = sm_pool.tile([H, wv], F32, name="wc_w", tag="wc_w")
                nc.vector.tensor_scalar(wc_w[:], mc1T[:],
                                        wp[:, W_HC1(i):W_HC1(i) + 1].bitcast(F32),
                                        None, OP.mult)
                wp_w = sm_pool.tile([H, wv], F32, name="wp_w", tag="wp_w")
                nc.vector.tensor_scalar(wp_w[:], mp1T[:],
                                        wp[:, W_HP1(i):W_HP1(i) + 1].bitcast(F32),
                                        None, OP.mult)

                qcw = sm_pool.tile([H, wv * NC], F32R, name="qcw", tag="qcw")
                esum = sm_pool.tile([1, wv], F32, name="esum", tag="esum")
                pfu = sm_pool.tile([wv, H], F32, name="pfu", tag="pfu")
                pf_stage = sm_pool.tile([H, wv * H], F32, name="pf_stage", tag="pf_stage", bufs=1)

                # ----- per graph heavy chain -----
                for j in range(wv):
                    g = gs + j
                    ps_p1 = ps_big.tile([H, NP], F32, name="big", tag="big")
                    nc.tensor.matmul(ps_p1[:, 0:512], wp[:, W_P2C(i):W_P2C(i) + H],
                                     peT[j][:, 0:512], start=True, stop=True)
                    nc.tensor.matmul(ps_p1[:, 512:1024], wp[:, W_P2C(i):W_P2C(i) + H],
                                     peT[j][:, 512:1024], start=True, stop=True)
                    ppreT = st_pool.tile([H, NP], F32, name="ppreT", tag="ppreT")
                    nc.scalar.activation(ppreT[:], ps_p1[:], A.Tanh,
                                         bias=bp[:, B_P2C(i):B_P2C(i) + 1])
                    ps_p2 = ps_big.tile([H, NP], F32, name="big", tag="big")
                    nc.tensor.matmul(ps_p2[:, 0:512], wp[:, W_HP0(i):W_HP0(i) + H],
                                     peT[j][:, 0:512], start=True, stop=True)
                    nc.tensor.matmul(ps_p2[:, 512:1024], wp[:, W_HP0(i):W_HP0(i) + H],
                                     peT[j][:, 512:1024], start=True, stop=True)
                    hp0T = st_pool.tile([H, NP], F32, name="hp0T", tag="hp0T")
                    nc.scalar.activation(hp0T[:], ps_p2[:], A.Tanh,
                                         bias=bp[:, B_HP0(i):B_HP0(i) + 1])

                    for half in range(2):
                        psq3 = ps_t.tile([H, 512], F32, name="pst", tag="pst")
                        for k in range(4):
                            ch = half * 4 + k
                            nc.tensor.transpose(psq3[:, k * 128:(k + 1) * 128],
                                                ppreT[:, ch * 128:(ch + 1) * 128], ident[:])
                        nc.vector.tensor_copy(
                            ppe[j][:, half * 4:(half + 1) * 4, 0:128],
                            psq3[:].rearrange("h (k c) -> h k c", k=4))

                    ps_cp2 = ps_big.tile([H, NP], F32, name="big", tag="big")
                    qb = (j % 2) * 64
                    lhs_cp = cpre_nm[qb:qb + 64, j // 2, :]
                    nc.tensor.matmul(ps_cp2[:, 0:512], lhs_cp, pair[j][:, 0:512],
                                     start=True, stop=True)
                    nc.tensor.matmul(ps_cp2[:, 512:1024], lhs_cp, pair[j][:, 512:1024],
                                     start=True, stop=True)

                    qwT = st_pool.tile([H, NP], F32R, name="qwT", tag="qwT")
                    nc.vector.scalar_tensor_tensor(qwT[:], ps_cp2[:], wp_w[:, j:j + 1],
                                                   hp0T[:], OP.mult, OP.mult)
                    ps_s = ps_big.tile([H, NP], F32, name="big", tag="big")
                    nc.tensor.matmul(ps_s[0:1, 0:512], ones_r[:, 0:1], qwT[:, 0:512],
                                     start=True, stop=True)
                    nc.tensor.matmul(ps_s[0:1, 512:1024], ones_r[:, 0:1], qwT[:, 512:1024],
                                     start=True, stop=True)
                    e_row = st_pool.tile([1, NP], F32, name="e_row", tag="e_row")
                    nc.scalar.activation(e_row[:], ps_s[0:1, :], A.Exp,
                                         accum_out=esum[0:1, j:j + 1])
                    s_dr = dr_pool.tile([NP], F32, name="s_dr", tag="s_dr")
                    nc.sync.dma_start(out=s_dr[:], in_=e_row[:])
                    if bisect:
                        nc.sync.dma_start(out=pwe[j][:, :, 64].bitcast(F32),
                                          in_=s_dr[:].rearrange("(p c) -> p c", c=NCHUNK))
                    else:
                        nc.sync.dma_start(out=pwe[j][:, :, 64].bitcast(F32),
                                          in_=s_dr[:].rearrange("(c p) -> p c", p=128))

                    ps_cc = ps_g.tile([65, 256], F32, name="psx", tag="psg2")
                    for k in range(NCHUNK):
                        nc.tensor.matmul(ps_cc[:], pwe[j][:, k, :], ppe[j][:, k, :],
                                         start=(k == 0), stop=(k == NCHUNK - 1))
                    p2c = st_pool.tile([64, H], F32, name="p2c", tag="p2c")
                    nc.vector.tensor_copy(p2c[:], ps_cc[0:64, 0:128])
                    ps_tc = ps_t.tile([H, 512], F32, name="pst", tag="pst")
                    nc.tensor.transpose(ps_tc[:, 0:64], p2c[:], ident[0:64, 0:64])
                    nc.vector.scalar_tensor_tensor(qcw[:, j * NC:(j + 1) * NC],
                                                   ps_tc[:, 0:64], wc_w[:, j:j + 1],
                                                   hc0T[:, j * NC:(j + 1) * NC],
                                                   OP.mult, OP.mult)
                    nc.vector.tensor_copy(pf_stage[64:65, j * H:(j + 1) * H],
                                          ps_cc[64:65, 128:256])

                # ----- batched c softmax + cf + pf + GRU -----
                ps_sc = psg2()
                nc.tensor.matmul(ps_sc[0:1, 0:wv * NC], ones_r[:, 0:1], qcw[:],
                                 start=True, stop=True)
                sc_rowt = sm_pool.tile([1, wv * NC], F32, name="sc_rowt", tag="sc_rowt",
                                       bufs=1)
                nc.scalar.activation(sc_rowt[:], ps_sc[0:1, 0:wv * NC], A.Copy)
                sc_dr = dr_pool.tile([G * NC], F32, name="sc_dr", tag="sc_dr")
                nc.sync.dma_start(out=sc_dr[0:wv * NC], in_=sc_rowt[:])
                sc_nm = sm_pool.tile([wv, NC], F32, name="sc_nm", tag="sc_nm")
                nc.sync.dma_start(out=sc_nm[:],
                                  in_=sc_dr[0:wv * NC].rearrange("(g c) -> g c", g=wv))  # contiguous
                negmax = sm_pool.tile([wv, 1], F32, name="negmax", tag="negmax")
                nc.vector.tensor_reduce(negmax[:], sc_nm[:], AX.X, OP.max, negate=True)
                eac = sm_pool.tile([wv, NC], F32, name="eac", tag="eac")
                sumec = sm_pool.tile([wv, 1], F32, name="sumec", tag="sumec")
                nc.scalar.activation(eac[:], sc_nm[:], A.Exp, bias=negmax[:],
                                     accum_out=sumec[:])
                rec_c = sm_pool.tile([wv, 1], F32, name="rec_c", tag="rec_c")
                nc.vector.reciprocal(rec_c[:], sumec[:])
                ac_nm = sm_pool.tile([wv, NC], F32, name="ac_nm", tag="ac_nm")
                nc.vector.tensor_scalar(ac_nm[:], eac[:], rec_c[:], None, OP.mult)
                # transpose into both parity halves
                ps_at = psg()
                nc.tensor.transpose(ps_at[0:NC, 0:wv], ac_nm[:], ident[0:wv, 0:wv])
                ac_stage = sm_pool.tile([NC, wv], F32, name="ac_stage", tag="ac_stage")
                nc.vector.tensor_copy(ac_stage[:], ps_at[0:NC, 0:wv])
                acT2 = sm_pool.tile([H, wv], F32R, name="acT2", tag="acT2")
                nc.vector.tensor_copy(acT2[0:NC, :], ac_stage[:])
                nc.sync.dma_start(out=acT2[64:128, :].bitcast(F32), in_=ac_stage[:])

                ps_cf = psg2()
                for j in range(wv):
                    g = gs + j
                    hb = (g % 2) * 64
                    nc.tensor.matmul(ps_cf[0:1, j * H:(j + 1) * H],
                                     acT2[hb:hb + 64, j:j + 1],
                                     ce_nm[hb:hb + 64, g // 2, :], start=True, stop=True)
                cf_row = sm_pool.tile([1, wv * H], F32, name="cf_row", tag="cf_row", bufs=1)
                nc.vector.tensor_copy(cf_row[:], ps_cf[0:1, 0:wv * H])
                cf_dr = dr_pool.tile([G * H], F32, name="cf_dr", tag="cf_dr")
                nc.sync.dma_start(out=cf_dr[0:wv * H], in_=cf_row[:])
                nc.sync.dma_start(out=xcf[:],
                                  in_=cf_dr[0:wv * H].rearrange("(j h) -> j h", j=wv))  # contiguous

                pf_dr = dr_pool.tile([G * H], F32, name="pf_dr", tag="pf_dr")
                nc.sync.dma_start(out=pf_dr[0:wv * H], in_=pf_stage[64:65, :])
                nc.sync.dma_start(out=pfu[:],
                                  in_=pf_dr[0:wv * H].rearrange("(j h) -> j h", j=wv))
                es_dr = dr_pool.tile([G], F32, name="es_dr", tag="es_dr")
                nc.sync.dma_start(out=es_dr[0:wv], in_=esum[:])
                esum_nm = sm_pool.tile([wv, 1], F32, name="esum_nm", tag="esum_nm")
                nc.sync.dma_start(out=esum_nm[:], in_=es_dr[0:wv].rearrange("(j a) -> j a", j=wv))
                esum2 = sm_pool.tile([wv, 1], F32, name="esum2", tag="esum2")
                nc.vector.tensor_scalar(esum2[:], esum_nm[:], 1e-6, None, OP.add)
                rec_p = sm_pool.tile([wv, 1], F32, name="rec_p", tag="rec_p")
                nc.vector.reciprocal(rec_p[:], esum2[:])
                nc.vector.tensor_scalar(pfn[:], pfu[:], rec_p[:], None, OP.mult)

                xw = sm_pool.tile([wv, H], F32, name="xw", tag="xw")
                nc.vector.tensor_mul(xw[:], xcf[:], pfn[:].bitcast(F32))
                ps_xt = psg()
                nc.tensor.transpose(ps_xt[0:H, 0:wv], xw[:], ident[0:wv, 0:wv])
                xT = sm_pool.tile([H, wv], F32R, name="xT", tag="xT")
                nc.vector.tensor_copy(xT[:], ps_xt[0:H, 0:wv])
                ps_gi = psg()
                nc.tensor.matmul(ps_gi[0:wv, 0:384], xT[:], wp[:, W_IH:W_IH + 384],
                                 start=True, stop=True)
                ps_gh = psg2()
                nc.tensor.matmul(ps_gh[0:wv, 0:384], mT[:], wp[:, W_HH:W_HH + 384],
                                 start=True, stop=True)
                gi = sm_pool.tile([wv, 384], F32, name="gi", tag="gi")
                nc.vector.tensor_add(gi[:], ps_gi[0:wv, 0:384], bg[0:wv, BG_IH:BG_IH + 384])
                gh = sm_pool.tile([wv, 384], F32, name="gh", tag="gh")
                nc.vector.tensor_add(gh[:], ps_gh[0:wv, 0:384], bg[0:wv, BG_HH:BG_HH + 384])
                rz_pre = sm_pool.tile([wv, 256], F32, name="rz_pre", tag="rz_pre", bufs=1)
                nc.vector.tensor_add(rz_pre[:], gi[:, 0:256], gh[:, 0:256])
                rz_t = sm_pool.tile([wv, 256], F32, name="rz_t", tag="rz_t")
                nc.scalar.activation(rz_t[:], rz_pre[:], A.Tanh, scale=0.5)
                rz = sm_pool.tile([wv, 256], F32, name="rz", tag="rz")
                nc.vector.tensor_scalar(rz[:], rz_t[:], 0.5, 0.5, OP.mult, OP.add)
                n_pre = sm_pool.tile([wv, H], F32, name="n_pre", tag="n_pre", bufs=1)
                nc.vector.tensor_mul(n_pre[:], rz[:, 0:H], gh[:, 256:384])
                n_pre2 = sm_pool.tile([wv, H], F32, name="n_pre2", tag="n_pre2", bufs=1)
                nc.vector.tensor_add(n_pre2[:], n_pre[:], gi[:, 256:384])
                n_t = sm_pool.tile([wv, H], F32, name="n_t", tag="n_t")
                nc.scalar.activation(n_t[:], n_pre2[:], A.Tanh)
                dmn = sm_pool.tile([wv, H], F32, name="dmn", tag="dmn", bufs=1)
                nc.vector.tensor_sub(dmn[:], m_nm[:], n_t[:])
                zd = sm_pool.tile([wv, H], F32, name="zd", tag="zd", bufs=1)
                nc.vector.tensor_mul(zd[:], rz[:, H:256], dmn[:])
                m_nm = sm_pool.tile([wv, H], F32, name="m_nm", tag="m_nm")
                nc.vector.tensor_add(m_nm[:], n_t[:], zd[:])
                ps_mT = psg()
                nc.tensor.transpose(ps_mT[0:H, 0:wv], m_nm[:], ident[0:wv, 0:wv])
                mT = sm_pool.tile([H, wv], F32R, name="mT", tag="mT")
                nc.vector.tensor_copy(mT[:], ps_mT[0:H, 0:wv])

            # ----- head -----
            acf = sm_pool.tile([wv, 2 * H], F32R, name="acf", tag="acf")
            nc.vector.tensor_copy(acf[:, 0:H], xcf[:])
            nc.vector.tensor_copy(acf[:, H:2 * H], sf_w[:])
            for j in range(wv):
                g = gs + j
                pfr = st_pool.tile([1, H], F32R, name="pfr", tag="pfr")
                nc.sync.dma_start(out=pfr[:], in_=pfn[j:j + 1, :])
                acfr = st_pool.tile([1, 2 * H], F32R, name="acfr", tag="acfr")
                nc.sync.dma_start(out=acfr[:], in_=acf[j:j + 1, :])
                ps_o = ps_g.tile([H, 256], F32, name="psx", tag="psg2")
                nc.tensor.matmul(ps_o[:], pfr[:], acfr[:],
                                 start=True, stop=True)
                gk = st_pool.tile([H, 2 * H], F32, name="gk", tag="gk")
                act_lrelu(gk[:], ps_o[:])
                gkw = st_pool.tile([H, 2 * H], F32, name="gkw", tag="gkw")
                nc.vector.scalar_tensor_tensor(gkw[:], gk[:], 1.0, w2t[:],
                                               OP.mult, OP.mult,
                                               accum_out=partials[:, g:g + 1])

        # ---------- output ----------
        ps_fin = ps_g.tile([G, 256], F32, name="psx", tag="psg2")
        nc.tensor.matmul(ps_fin[0:G, 0:2], partials[:], ones_r[:], start=True, stop=True)
        ofin = gl_pool.tile([G, 1], F32, name="ofin", tag="ofin")
        nc.vector.tensor_scalar(ofin[:], ps_fin[0:G, 0:1], float(b_out_val), None, OP.add)
        nc.sync.dma_start(out=out_d[:], in_=ofin[:])

    if split:
        _split_waits(nc)
    return nc


def kernel(**inputs) -> np.ndarray:
    f = {k: np.asarray(v) for k, v in inputs.items()}
    f = {k: (v.astype(np.float32) if v.dtype != np.int32 else v) for k, v in f.items()}

    wblocks = [f['W_pc'], f['W_pp'], f['W_caff'], f['W_paff'], f['W_saff']]
    wblocks += [f['W_c2p'][i] for i in range(D)]
    wblocks += [f['W_hc0'][i] for i in range(D)]
    wblocks += [f['W_p2c'][i] for i in range(D)]
    wblocks += [f['W_hp0'][i] for i in range(D)]
    wblocks += [np.concatenate([f['W_mc1'][i], f['W_mp1'][i]], axis=1) for i in range(D)]
    wblocks += [f['W_ih'], f['W_hh']]
    wblocks += [f['W_hc1'][i] for i in range(D)]
    wblocks += [f['W_hp1'][i] for i in range(D)]
    wpack = np.ascontiguousarray(np.concatenate(wblocks, axis=1), dtype=np.float32)
    assert wpack.shape == (H, WCOLS), wpack.shape

    bcols = [f['b_pp'], f['b_paff'], f['b_pc'], f['b_caff']]
    bcols += [f['b_c2p'][i] for i in range(D)]
    bcols += [f['b_hc0'][i] for i in range(D)]
    bcols += [f['b_p2c'][i] for i in range(D)]
    bcols += [f['b_hp0'][i] for i in range(D)]
    bpack = np.stack(bcols, axis=1).astype(np.float32)
    assert bpack.shape == (H, BCOLS)

    bgparts = [np.tile(f['b_saff'][None, :], (G, 1))]
    bgparts += [np.tile(np.concatenate([f['b_mc1'][i], f['b_mp1'][i]])[None, :], (G, 1))
                for i in range(D)]
    bgparts += [np.tile(f['b_ih'][None, :], (G, 1)), np.tile(f['b_hh'][None, :], (G, 1))]
    bg16 = np.concatenate(bgparts, axis=1).astype(np.float32)
    assert bg16.shape == (G, BGCOLS)

    w2t = np.ascontiguousarray(f['W_out'].reshape(2 * H, H).T, dtype=np.float32)
    b_out_val = float(f['b_out'][0])

    key = ('nc', b_out_val)
    if key not in _CACHE:
        _CACHE[key] = _build(b_out_val)
    nc = _CACHE[key]

    comp = f['comp_feature'].reshape(NCORES, G, NC, H)
    prot = f['prot_feature'].reshape(NCORES, G, NP, H)
    gomp = f['gomp_feature'].reshape(NCORES, G, H)
    in_maps = []
    for c in range(NCORES):
        in_maps.append({
            "protT": np.ascontiguousarray(prot[c].transpose(2, 0, 1).reshape(H, G * NP)),
            "compT": np.ascontiguousarray(comp[c].transpose(2, 0, 1).reshape(H, G * NC)),
            "gompT": np.ascontiguousarray(gomp[c].T),
            "wpack": wpack, "bpack": bpack, "bg16": bg16, "w2t": w2t,
        })

    global LAST_EXEC_NS
    try:
        r = run_bass_kernel_spmd(nc, in_maps, list(range(NCORES)), trace=TRACE)
    except ModuleNotFoundError:
        r = run_bass_kernel_spmd(nc, in_maps, list(range(NCORES)))
    if getattr(r, "exec_time_ns", None):
        LAST_EXEC_NS = r.exec_time_ns
    res = r.results
    return np.concatenate([res[c]["out"] for c in range(NCORES)], axis=0)



# revision 19
# speedup vs baseline: 1.0172x; 1.0172x over previous
"""Trainium2 Bass kernel for AffinityNeuralNetworkMONN (gnn_message_passing).

Sharding: data-parallel over B=128 graphs -> 8 NeuronCores x 16 graphs.
Inside a core, graphs are processed in waves; per-graph heavy tensors use
a [H=128 partitions, nodes free] (T) layout so ACT bias/scale fuse per
partition; node-contraction operands are built NM via PE transposes.
All matmuls run as float32r (full-rate at N>=256). Softmax score rows are
reduced across partitions on GPSIMD and scattered to node-major via DMA.
Only tanh/exp/prelu ACT functions are used (one table set, no reloads).
"""
import sys
for p in ("/opt/trn_rl_repo", "/root/.axon_site/_ro/trn_rl_repo"):
    if p not in sys.path:
        sys.path.insert(0, p)

import numpy as np
import os
from contextlib import ExitStack

import concourse.bass as bass
import concourse.tile as tile
from concourse import mybir, masks
from concourse import bass_isa
from concourse.bass_utils import run_bass_kernel_spmd

F32 = mybir.dt.float32
F32R = mybir.dt.float32r
A = mybir.ActivationFunctionType
OP = mybir.AluOpType
AX = mybir.AxisListType

NCORES = 8
B, NC, NP, H, D = 128, 64, 1024, 128, 3
G = B // NCORES            # graphs per core = 16
WAVES = [2] * 8            # wave sizes (sum = 16); wv<=2 verified on HW
if os.environ.get("KWAVES"):
    WAVES = [int(x) for x in os.environ["KWAVES"].split(",")]
NCHUNK = NP // 128         # 8 p-chunks per graph

W_PC, W_PP, W_CAFF, W_PAFF, W_SAFF = 0, 128, 256, 384, 512
def W_C2P(i): return 640 + i * 128
def W_HC0(i): return 1024 + i * 128
def W_P2C(i): return 1408 + i * 128
def W_HP0(i): return 1792 + i * 128
def W_MCP(i): return 2176 + i * 256
W_IH, W_HH = 2944, 3328
def W_HC1(i): return 3712 + i
def W_HP1(i): return 3715 + i
WCOLS = 3718

B_PP, B_PAFF, B_PC, B_CAFF = 0, 1, 2, 3
def B_C2P(i): return 4 + i
def B_HC0(i): return 7 + i
def B_P2C(i): return 10 + i
def B_HP0(i): return 13 + i
BCOLS = 16

BG_SAFF = 0
def BG_MCP(i): return 128 + i * 256
BG_IH, BG_HH = 896, 1280
BGCOLS = 1664

_CACHE = {}
TRACE = False
LAST_EXEC_NS = None


def _split_waits(nc, keep=1):
    """walrus allows very few attached sync-waits per instruction (1 for the
    f32 self-loading matmul struct). Hoist excess waits into standalone
    EventSemaphore instructions right before the over-subscribed one."""
    for fn in nc.m.functions:
        for blk in fn.blocks:
            out = []
            for ins in blk.instructions:
                si = ins.sync_info
                if si is not None and si.on_wait and len(si.on_wait) > keep:
                    waits = list(si.on_wait)
                    for jj, w in enumerate(waits[:-keep]):
                        ev = mybir.InstNoOp(
                            name=f"{ins.name}-wsplit{jj}",
                            sync_info=mybir.SyncInfo(on_wait=[w], on_update=[]),
                            bass_nofuse=True)
                        ev.engine = ins.engine
                        out.append(ev)
                    si.on_wait = waits[-keep:]
                    ins.sync_info = si
                out.append(ins)
            blk.instructions = out


def _build(b_out_val: float, split: bool = True, sim_compat: bool = False):
    bisect = os.environ.get("KBISECT", "0") == "1"
    nc = bass.Bass()
    protT_d = nc.dram_tensor("protT", [H, G * NP], F32, kind="ExternalInput")
    compT_d = nc.dram_tensor("compT", [H, G * NC], F32, kind="ExternalInput")
    gompT_d = nc.dram_tensor("gompT", [H, G], F32, kind="ExternalInput")
    wpack_d = nc.dram_tensor("wpack", [H, WCOLS], F32, kind="ExternalInput")
    bpack_d = nc.dram_tensor("bpack", [H, BCOLS], F32, kind="ExternalInput")
    bg16_d = nc.dram_tensor("bg16", [G, BGCOLS], F32, kind="ExternalInput")
    w2t_d = nc.dram_tensor("w2t", [H, 2 * H], F32, kind="ExternalInput")
    out_d = nc.dram_tensor("out", [G, 1], F32, kind="ExternalOutput")

    with tile.TileContext(nc) as tc, ExitStack() as ctx:
        gl_pool = ctx.enter_context(tc.tile_pool(name="globals", bufs=1))
        per_pool = ctx.enter_context(tc.tile_pool(name="persist", bufs=1))
        st_pool = ctx.enter_context(tc.tile_pool(name="stream", bufs=2))
        sm_pool = ctx.enter_context(tc.tile_pool(name="small", bufs=2))
        ps_big = ctx.enter_context(tc.tile_pool(name="psBig", bufs=2, space="PSUM"))
        ps_t = ctx.enter_context(tc.tile_pool(name="psT", bufs=2, space="PSUM"))
        ps_g = ctx.enter_context(tc.tile_pool(name="psG", bufs=1, space="PSUM"))
        dr_pool = ctx.enter_context(tc.tile_pool(name="dram", bufs=2, space="DRAM"))

        def psg():
            return ps_g.tile([H, 512], F32, name="psg", tag="psg")

        def psg2():
            return ps_g.tile([H, 512], F32, name="psg2", tag="psg2")

        def act_lrelu(dst, src_ps, bias=0.0, accum_out=None):
            if not sim_compat:
                nc.scalar.activation(dst, src_ps, A.Prelu, bias=bias, alpha=0.1,
                                     accum_out=accum_out)
            else:
                shp = [dst.shape[0], int(np.prod(dst.shape[1:]))]
                t1 = sm_pool.tile(shp, F32, name="lr1", tag="lr_t1", bufs=1)
                t2 = sm_pool.tile(shp, F32, name="lr2", tag="lr_t2", bufs=1)
                nb = bias if isinstance(bias, float) else None
                nc.scalar.activation(t1[:], src_ps, A.Relu, bias=bias)
                if nb is None:
                    negb = sm_pool.tile([dst.shape[0], 1], F32, name="lrnb",
                                        tag="lr_nb", bufs=1)
                    nc.vector.tensor_scalar(negb[:], bias, -1.0, None, OP.mult)
                    nc.scalar.activation(t2[:], src_ps, A.Relu, scale=-1.0, bias=negb[:])
                else:
                    nc.scalar.activation(t2[:], src_ps, A.Relu, scale=-1.0, bias=-nb)
                nc.vector.scalar_tensor_tensor(dst, t2[:], -0.1, t1[:],
                                               OP.mult, OP.add, accum_out=accum_out)

        # ---------- preamble ----------
        wp = gl_pool.tile([H, WCOLS], F32R, name="wp", tag="wp")
        bp = gl_pool.tile([H, BCOLS], F32, name="bp", tag="bp")
        bg = gl_pool.tile([G, BGCOLS], F32, name="bg", tag="bg")
        w2t = gl_pool.tile([H, 2 * H], F32, name="w2t", tag="w2t")
        compT = gl_pool.tile([H, G * NC], F32R, name="compT", tag="compT")
        gompT = gl_pool.tile([H, G], F32R, name="gompT", tag="gompT")
        nc.sync.dma_start(out=wp[:], in_=wpack_d[:].bitcast(F32R))
        nc.sync.dma_start(out=bp[:], in_=bpack_d[:])
        nc.sync.dma_start(out=bg[:], in_=bg16_d[:])
        nc.sync.dma_start(out=w2t[:], in_=w2t_d[:])
        nc.sync.dma_start(out=compT[:], in_=compT_d[:].bitcast(F32R))
        nc.sync.dma_start(out=gompT[:], in_=gompT_d[:].bitcast(F32R))

        ident = gl_pool.tile([H, H], F32, name="ident", tag="ident")
        masks.make_identity(nc, ident[:])
        ones_r = gl_pool.tile([H, 2], F32R, name="ones_r", tag="ones_r")
        nc.vector.memset(ones_r[:].bitcast(F32), 1.0)
        identr = ident[:].bitcast(F32R)

        ceT = gl_pool.tile([H, G * NC], F32R, name="ceT", tag="ceT")
        pcT = gl_pool.tile([H, G * NC], F32R, name="pcT", tag="pcT")
        for (dst, wcol, bcol) in ((ceT, W_CAFF, B_CAFF), (pcT, W_PC, B_PC)):
            pscc = ps_big.tile([H, G * NC], F32, name="big", tag="big")
            nc.tensor.matmul(pscc[:, 0:512], wp[:, wcol:wcol + H], compT[:, 0:512],
                             start=True, stop=True)
            nc.tensor.matmul(pscc[:, 512:1024], wp[:, wcol:wcol + H], compT[:, 512:1024],
                             start=True, stop=True)
            act_lrelu(dst[:], pscc[:], bias=bp[:, bcol:bcol + 1])

        # CE_NM [128, 8, 128]: pair-transposed ce (abs graphs 2k, 2k+1 stacked)
        ce_nm = gl_pool.tile([H, 8, H], F32R, name="ce_nm", tag="ce_nm")
        for half in range(2):
            pst = ps_t.tile([H, 512], F32, name="pst", tag="pst")
            for k in range(4):
                pr = half * 4 + k
                nc.tensor.transpose(pst[:, k * 128:(k + 1) * 128],
                                    ceT[:, pr * 128:(pr + 1) * 128].bitcast(F32), ident[:])
            nc.vector.tensor_copy(ce_nm[:, half * 4:(half + 1) * 4, :],
                                  pst[:].rearrange("h (k c) -> h k c", k=4))

        cesum = gl_pool.tile([H, G], F32, name="cesum", tag="cesum")
        nc.vector.tensor_reduce(cesum[:],
                                ceT[:].bitcast(F32).rearrange("h (g c) -> h g c", g=G),
                                AX.X, OP.add)
        peacc = gl_pool.tile([H, G], F32, name="peacc", tag="peacc")
        partials = gl_pool.tile([H, G], F32R, name="partials", tag="partials")

        # ---------- waves ----------
        g0 = 0
        for wv in WAVES:
            gs, ge = g0, g0 + wv
            g0 = ge

            peT = [per_pool.tile([H, NP], F32R, name=f"peT{j}", tag=f"peT{j}") for j in range(wv)]
            pairslab = [per_pool.tile([H, NP], F32R, name=f"pairs{q}", tag=f"pairs{q}")
                        for q in range((wv + 1) // 2)]
            pair = [pairslab[j // 2][(j % 2) * 64:(j % 2) * 64 + 64, :] for j in range(wv)]
            pwe = [per_pool.tile([H, NCHUNK, 65], F32R, name=f"pwe{j}", tag=f"pwe{j}") for j in range(wv)]
            ppe = [per_pool.tile([H, NCHUNK, 256], F32R, name=f"ppe{j}", tag=f"ppe{j}") for j in range(wv)]

            # ----- phase A -----
            for j in range(wv):
                g = gs + j
                protT = st_pool.tile([H, NP], F32R, name="protT", tag="protT")
                nc.sync.dma_start(out=protT[:],
                                  in_=protT_d[:, g * NP:(g + 1) * NP].bitcast(F32R))

                ps_pp = ps_big.tile([H, NP], F32, name="big", tag="big")
                nc.tensor.matmul(ps_pp[:, 0:512], wp[:, W_PP:W_PP + H], protT[:, 0:512],
                                 start=True, stop=True)
                nc.tensor.matmul(ps_pp[:, 512:1024], wp[:, W_PP:W_PP + H],
                                 protT[:, 512:1024], start=True, stop=True)
                ppT = st_pool.tile([H, NP], F32R, name="ppT", tag="ppT")
                act_lrelu(ppT[:], ps_pp[:], bias=bp[:, B_PP:B_PP + 1])

                ps_pe = ps_big.tile([H, NP], F32, name="big", tag="big")
                nc.tensor.matmul(ps_pe[:, 0:512], wp[:, W_PAFF:W_PAFF + H], protT[:, 0:512],
                                 start=True, stop=True)
                nc.tensor.matmul(ps_pe[:, 512:1024], wp[:, W_PAFF:W_PAFF + H],
                                 protT[:, 512:1024], start=True, stop=True)
                act_lrelu(peT[j][:], ps_pe[:], bias=bp[:, B_PAFF:B_PAFF + 1],
                          accum_out=peacc[:, g:g + 1])

                # pairwise = sigmoid(pc @ pp^T) = 0.5 + 0.5*tanh(z/2)
                hb = (j % 2) * 64
                ps_pw = ps_big.tile([H, NP], F32, name="big", tag="big")
                nc.tensor.matmul(ps_pw[0:64, 0:512], pcT[:, g * NC:(g + 1) * NC],
                                 ppT[:, 0:512], start=True, stop=True)
                nc.tensor.matmul(ps_pw[0:64, 512:1024], pcT[:, g * NC:(g + 1) * NC],
                                 ppT[:, 512:1024], start=True, stop=True)
                pw_t = st_pool.tile([H, NP], F32, name="pw_t", tag="pw_t")
                nc.scalar.activation(pw_t[0:64, :], ps_pw[0:64, :], A.Tanh, scale=0.5)
                if hb == 0:
                    nc.vector.tensor_scalar(pair[j], pw_t[0:64, :], 0.5, 0.5,
                                            OP.mult, OP.add)
                else:
                    pair_st = st_pool.tile([64, NP], F32, name="pair_st", tag="pair_st")
                    nc.vector.tensor_scalar(pair_st[:], pw_t[0:64, :], 0.5, 0.5,
                                            OP.mult, OP.add)
                    nc.sync.dma_start(out=pair[j].bitcast(F32), in_=pair_st[:])

                # pairwiseT -> pwe cols 0:64
                for half in range(2):
                    pstp = ps_t.tile([H, 512], F32, name="pst", tag="pst")
                    for k in range(4):
                        ch = half * 4 + k
                        nc.tensor.transpose(pstp[:, k * 128:k * 128 + 64],
                                            pair[j][:, ch * 128:(ch + 1) * 128].bitcast(F32),
                                            ident[hb:hb + 64, hb:hb + 64])
                    nc.vector.tensor_copy(
                        pwe[j][:, half * 4:(half + 1) * 4, 0:64],
                        pstp[:].rearrange("h (k c) -> h k c", k=4)[:, :, 0:64])

                # peT transposes -> ppe cols 128:256 (pe_NM)
                for half in range(2):
                    psq = ps_t.tile([H, 512], F32, name="pst", tag="pst")
                    for k in range(4):
                        ch = half * 4 + k
                        nc.tensor.transpose(psq[:, k * 128:(k + 1) * 128],
                                            peT[j][:, ch * 128:(ch + 1) * 128].bitcast(F32),
                                            ident[:])
                    nc.vector.tensor_copy(
                        ppe[j][:, half * 4:(half + 1) * 4, 128:256],
                        psq[:].rearrange("h (k c) -> h k c", k=4))

            # sf for this wave: lrelu(gomp @ W_saff + b_saff)
            ps_sf = psg()
            nc.tensor.matmul(ps_sf[0:wv, 0:256], gompT[:, gs:ge],
                             wp[:, W_SAFF:W_SAFF + 256], start=True, stop=True)
            sf_pre = sm_pool.tile([wv, H], F32, name="sf_pre", tag="sf_pre", bufs=1)
            nc.vector.tensor_add(sf_pre[:], ps_sf[0:wv, 0:H], bg[0:wv, BG_SAFF:BG_SAFF + H])
            sf_w = sm_pool.tile([wv, H], F32, name="sf_w", tag="sf_w")
            act_lrelu(sf_w[:], sf_pre[:])

            # m0
            mT = sm_pool.tile([H, wv], F32R, name="mT", tag="mT")
            nc.vector.scalar_tensor_tensor(mT[:], cesum[:, gs:ge], 1.0 / (NC * NP),
                                           peacc[:, gs:ge], OP.mult, OP.mult)
            ps_m0 = psg()
            nc.tensor.transpose(ps_m0[0:wv, 0:H], mT[:].bitcast(F32), ident[:])
            m_nm = sm_pool.tile([wv, H], F32, name="m_nm", tag="m_nm")
            nc.vector.tensor_copy(m_nm[:], ps_m0[0:wv, 0:H])

            xcf = sm_pool.tile([wv, H], F32, name="xcf", tag="xcf")
            pfn = sm_pool.tile([wv, H], F32R, name="pfn", tag="pfn")

            # ----- phase B: D iterations -----
            for i in range(D):
                csl = slice(gs * NC, ge * NC)
                ps_cp = ps_big.tile([H, wv * NC], F32, name="big", tag="big")
                nc.tensor.matmul(ps_cp[:], wp[:, W_C2P(i):W_C2P(i) + H], ceT[:, csl],
                                 start=True, stop=True)
                cpreT = sm_pool.tile([H, wv * NC], F32, name="cpreT", tag="cpreT")
                nc.scalar.activation(cpreT[:], ps_cp[:], A.Tanh,
                                     bias=bp[:, B_C2P(i):B_C2P(i) + 1])
                ps_h0 = ps_big.tile([H, wv * NC], F32, name="big", tag="big")
                nc.tensor.matmul(ps_h0[:], wp[:, W_HC0(i):W_HC0(i) + H], ceT[:, csl],
                                 start=True, stop=True)
                hc0T = sm_pool.tile([H, wv * NC], F32, name="hc0T", tag="hc0T")
                nc.scalar.activation(hc0T[:], ps_h0[:], A.Tanh,
                                     bias=bp[:, B_HC0(i):B_HC0(i) + 1])

                # c_pre_NM (graph-pair transposes; wave starts are odd-free:
                # waves [5,5,3] start at 0,5,10 -> pairs may straddle; use
                # per-graph 64-col transposes into fixed parity slots)
                cpre_nm = sm_pool.tile([H, 2, H], F32R, name="cpre_nm", tag="cpre_nm")
                psq2 = ps_t.tile([H, 512], F32, name="pst", tag="pst")
                for j in range(wv):
                    nc.tensor.transpose(psq2[0:64, j * 128:(j + 1) * 128],
                                        cpreT[:, j * 64:(j + 1) * 64], ident[:])
                # evens -> partitions 0:64 (DVE), odds -> 64:128 (DMA shifts partitions)
                ne, no = (wv + 1) // 2, wv // 2
                psq2v = psq2[0:64, 0:wv * 128].rearrange("c (j h) -> c j h", j=wv)
                nc.vector.tensor_copy(cpre_nm[0:64, 0:ne, :], psq2v[:, 0::2, :])
                cpre_odd = sm_pool.tile([64, 2, H], F32, name="cpre_odd", tag="cpre_odd", bufs=1)
                nc.vector.tensor_copy(cpre_odd[:, 0:no, :], psq2v[:, 1::2, :])
                nc.sync.dma_start(out=cpre_nm[64:128, 0:no, :].bitcast(F32),
                                  in_=cpre_odd[:, 0:no, :])

                # mc1/mp1 batched
                ps_mm = psg()
                nc.tensor.matmul(ps_mm[0:wv, 0:256], mT[:], wp[:, W_MCP(i):W_MCP(i) + 256],
                                 start=True, stop=True)
                mcp_pre = sm_pool.tile([wv, 256], F32, name="mcp_pre", tag="mcp_pre", bufs=1)
                nc.vector.tensor_add(mcp_pre[:], ps_mm[0:wv, 0:256],
                                     bg[0:wv, BG_MCP(i):BG_MCP(i) + 256])
                mcp = sm_pool.tile([wv, 256], F32, name="mcp", tag="mcp")
                nc.scalar.activation(mcp[:], mcp_pre[:], A.Tanh)
                ps_mt = psg()
                nc.tensor.transpose(ps_mt[0:H, 0:wv], mcp[:, 0:H], ident[0:wv, 0:wv])
                mc1T = sm_pool.tile([H, wv], F32, name="mc1T", tag="mc1T")
                nc.vector.tensor_copy(mc1T[:], ps_mt[0:H, 0:wv])
                ps_mt2 = psg()
                nc.tensor.transpose(ps_mt2[0:H, 0:wv], mcp[:, H:256], ident[0:wv, 0:wv])
                mp1T = sm_pool.tile([H, wv], F32, name="mp1T", tag="mp1T")
                nc.vector.tensor_copy(mp1T[:], ps_mt2[0:H, 0:wv])

                wc_w = sm_pool.tile([H, wv], F32, name="wc_w", tag="wc_w")
                nc.vector.tensor_scalar(wc_w[:], mc1T[:],
                                        wp[:, W_HC1(i):W_HC1(i) + 1].bitcast(F32),
                                        None, OP.mult)
                wp_w = sm_pool.tile([H, wv], F32, name="wp_w", tag="wp_w")
                nc.vector.tensor_scalar(wp_w[:], mp1T[:],
                                        wp[:, W_HP1(i):W_HP1(i) + 1].bitcast(F32),
                                        None, OP.mult)

                qcw = sm_pool.tile([H, wv * NC], F32R, name="qcw", tag="qcw")
                esum = sm_pool.tile([1, wv], F32, name="esum", tag="esum")
                pfu = sm_pool.tile([wv, H], F32, name="pfu", tag="pfu")
                pf_stage = sm_pool.tile([H, wv * H], F32, name="pf_stage", tag="pf_stage", bufs=1)

                # ----- per graph heavy chain (pass 1: through e-scatter) -----
                for j in range(wv):
                    g = gs + j
                    ps_p1 = ps_big.tile([H, NP], F32, name="big", tag="big")
                    nc.tensor.matmul(ps_p1[:, 0:512], wp[:, W_P2C(i):W_P2C(i) + H],
                                     peT[j][:, 0:512], start=True, stop=True)
                    nc.tensor.matmul(ps_p1[:, 512:1024], wp[:, W_P2C(i):W_P2C(i) + H],
                                     peT[j][:, 512:1024], start=True, stop=True)
                    ppreT = st_pool.tile([H, NP], F32, name="ppreT", tag="ppreT")
                    nc.scalar.activation(ppreT[:], ps_p1[:], A.Tanh,
                                         bias=bp[:, B_P2C(i):B_P2C(i) + 1])
                    ps_p2 = ps_big.tile([H, NP], F32, name="big", tag="big")
                    nc.tensor.matmul(ps_p2[:, 0:512], wp[:, W_HP0(i):W_HP0(i) + H],
                                     peT[j][:, 0:512], start=True, stop=True)
                    nc.tensor.matmul(ps_p2[:, 512:1024], wp[:, W_HP0(i):W_HP0(i) + H],
                                     peT[j][:, 512:1024], start=True, stop=True)
                    hp0T = st_pool.tile([H, NP], F32, name="hp0T", tag="hp0T")
                    nc.scalar.activation(hp0T[:], ps_p2[:], A.Tanh,
                                         bias=bp[:, B_HP0(i):B_HP0(i) + 1])

                    for half in range(2):
                        psq3 = ps_t.tile([H, 512], F32, name="pst", tag="pst")
                        for k in range(4):
                            ch = half * 4 + k
                            nc.tensor.transpose(psq3[:, k * 128:(k + 1) * 128],
                                                ppreT[:, ch * 128:(ch + 1) * 128], ident[:])
                        nc.vector.tensor_copy(
                            ppe[j][:, half * 4:(half + 1) * 4, 0:128],
                            psq3[:].rearrange("h (k c) -> h k c", k=4))

                    ps_cp2 = ps_big.tile([H, NP], F32, name="big", tag="big")
                    qb = (j % 2) * 64
                    lhs_cp = cpre_nm[qb:qb + 64, j // 2, :]
                    nc.tensor.matmul(ps_cp2[:, 0:512], lhs_cp, pair[j][:, 0:512],
                                     start=True, stop=True)
                    nc.tensor.matmul(ps_cp2[:, 512:1024], lhs_cp, pair[j][:, 512:1024],
                                     start=True, stop=True)

                    qwT = st_pool.tile([H, NP], F32R, name="qwT", tag="qwT")
                    nc.vector.scalar_tensor_tensor(qwT[:], ps_cp2[:], wp_w[:, j:j + 1],
                                                   hp0T[:], OP.mult, OP.mult)
                    ps_s = ps_big.tile([H, NP], F32, name="big", tag="big")
                    nc.tensor.matmul(ps_s[0:1, 0:512], ones_r[:, 0:1], qwT[:, 0:512],
                                     start=True, stop=True)
                    nc.tensor.matmul(ps_s[0:1, 512:1024], ones_r[:, 0:1], qwT[:, 512:1024],
                                     start=True, stop=True)
                    e_row = st_pool.tile([1, NP], F32, name="e_row", tag="e_row")
                    nc.scalar.activation(e_row[:], ps_s[0:1, :], A.Exp,
                                         accum_out=esum[0:1, j:j + 1])
                    s_dr = dr_pool.tile([NP], F32, name="s_dr", tag="s_dr")
                    nc.sync.dma_start(out=s_dr[:], in_=e_row[:])
                    if bisect:
                        nc.sync.dma_start(out=pwe[j][:, :, 64].bitcast(F32),
                                          in_=s_dr[:].rearrange("(p c) -> p c", c=NCHUNK))
                    else:
                        nc.sync.dma_start(out=pwe[j][:, :, 64].bitcast(F32),
                                          in_=s_dr[:].rearrange("(c p) -> p c", p=128))

                # ----- pass 2: node-contraction per graph -----
                for j in range(wv):
                    g = gs + j
                    ps_cc = ps_g.tile([65, 256], F32, name="psx", tag="psg2")
                    for k in range(NCHUNK):
                        nc.tensor.matmul(ps_cc[:], pwe[j][:, k, :], ppe[j][:, k, :],
                                         start=(k == 0), stop=(k == NCHUNK - 1))
                    p2c = st_pool.tile([64, H], F32, name="p2c", tag="p2c")
                    nc.vector.tensor_copy(p2c[:], ps_cc[0:64, 0:128])
                    ps_tc = ps_t.tile([H, 512], F32, name="pst", tag="pst")
                    nc.tensor.transpose(ps_tc[:, 0:64], p2c[:], ident[0:64, 0:64])
                    nc.vector.scalar_tensor_tensor(qcw[:, j * NC:(j + 1) * NC],
                                                   ps_tc[:, 0:64], wc_w[:, j:j + 1],
                                                   hc0T[:, j * NC:(j + 1) * NC],
                                                   OP.mult, OP.mult)
                    nc.vector.tensor_copy(pf_stage[64:65, j * H:(j + 1) * H],
                                          ps_cc[64:65, 128:256])

                # ----- batched c softmax + cf + pf + GRU -----
                ps_sc = psg2()
                nc.tensor.matmul(ps_sc[0:1, 0:wv * NC], ones_r[:, 0:1], qcw[:],
                                 start=True, stop=True)
                sc_rowt = sm_pool.tile([1, wv * NC], F32, name="sc_rowt", tag="sc_rowt",
                                       bufs=1)
                nc.scalar.activation(sc_rowt[:], ps_sc[0:1, 0:wv * NC], A.Copy)
                sc_dr = dr_pool.tile([G * NC], F32, name="sc_dr", tag="sc_dr")
                nc.sync.dma_start(out=sc_dr[0:wv * NC], in_=sc_rowt[:])
                sc_nm = sm_pool.tile([wv, NC], F32, name="sc_nm", tag="sc_nm")
                nc.sync.dma_start(out=sc_nm[:],
                                  in_=sc_dr[0:wv * NC].rearrange("(g c) -> g c", g=wv))  # contiguous
                negmax = sm_pool.tile([wv, 1], F32, name="negmax", tag="negmax")
                nc.vector.tensor_reduce(negmax[:], sc_nm[:], AX.X, OP.max, negate=True)
                eac = sm_pool.tile([wv, NC], F32, name="eac", tag="eac")
                sumec = sm_pool.tile([wv, 1], F32, name="sumec", tag="sumec")
                nc.scalar.activation(eac[:], sc_nm[:], A.Exp, bias=negmax[:],
                                     accum_out=sumec[:])
                rec_c = sm_pool.tile([wv, 1], F32, name="rec_c", tag="rec_c")
                nc.vector.reciprocal(rec_c[:], sumec[:])
                ac_nm = sm_pool.tile([wv, NC], F32, name="ac_nm", tag="ac_nm")
                nc.vector.tensor_scalar(ac_nm[:], eac[:], rec_c[:], None, OP.mult)
                # transpose into both parity halves
                ps_at = psg()
                nc.tensor.transpose(ps_at[0:NC, 0:wv], ac_nm[:], ident[0:wv, 0:wv])
                ac_stage = sm_pool.tile([NC, wv], F32, name="ac_stage", tag="ac_stage")
                nc.vector.tensor_copy(ac_stage[:], ps_at[0:NC, 0:wv])
                acT2 = sm_pool.tile([H, wv], F32R, name="acT2", tag="acT2")
                nc.vector.tensor_copy(acT2[0:NC, :], ac_stage[:])
                nc.sync.dma_start(out=acT2[64:128, :].bitcast(F32), in_=ac_stage[:])

                ps_cf = psg2()
                for j in range(wv):
                    g = gs + j
                    hb = (g % 2) * 64
                    nc.tensor.matmul(ps_cf[0:1, j * H:(j + 1) * H],
                                     acT2[hb:hb + 64, j:j + 1],
                                     ce_nm[hb:hb + 64, g // 2, :], start=True, stop=True)
                cf_row = sm_pool.tile([1, wv * H], F32, name="cf_row", tag="cf_row", bufs=1)
                nc.vector.tensor_copy(cf_row[:], ps_cf[0:1, 0:wv * H])
                cf_dr = dr_pool.tile([G * H], F32, name="cf_dr", tag="cf_dr")
                nc.sync.dma_start(out=cf_dr[0:wv * H], in_=cf_row[:])
                nc.sync.dma_start(out=xcf[:],
                                  in_=cf_dr[0:wv * H].rearrange("(j h) -> j h", j=wv))  # contiguous

                pf_dr = dr_pool.tile([G * H], F32, name="pf_dr", tag="pf_dr")
                nc.sync.dma_start(out=pf_dr[0:wv * H], in_=pf_stage[64:65, :])
                nc.sync.dma_start(out=pfu[:],
                                  in_=pf_dr[0:wv * H].rearrange("(j h) -> j h", j=wv))
                es_dr = dr_pool.tile([G], F32, name="es_dr", tag="es_dr")
                nc.sync.dma_start(out=es_dr[0:wv], in_=esum[:])
                esum_nm = sm_pool.tile([wv, 1], F32, name="esum_nm", tag="esum_nm")
                nc.sync.dma_start(out=esum_nm[:], in_=es_dr[0:wv].rearrange("(j a) -> j a", j=wv))
                esum2 = sm_pool.tile([wv, 1], F32, name="esum2", tag="esum2")
                nc.vector.tensor_scalar(esum2[:], esum_nm[:], 1e-6, None, OP.add)
                rec_p = sm_pool.tile([wv, 1], F32, name="rec_p", tag="rec_p")
                nc.vector.reciprocal(rec_p[:], esum2[:])
                nc.vector.tensor_scalar(pfn[:], pfu[:], rec_p[:], None, OP.mult)

                if i == D - 1:
                    continue
                xw = sm_pool.tile([wv, H], F32, name="xw", tag="xw")
                nc.vector.tensor_mul(xw[:], xcf[:], pfn[:].bitcast(F32))
                ps_xt = psg()
                nc.tensor.transpose(ps_xt[0:H, 0:wv], xw[:], ident[0:wv, 0:wv])
                xT = sm_pool.tile([H, wv], F32R, name="xT", tag="xT")
                nc.vector.tensor_copy(xT[:], ps_xt[0:H, 0:wv])
                ps_gi = psg()
                nc.tensor.matmul(ps_gi[0:wv, 0:384], xT[:], wp[:, W_IH:W_IH + 384],
                                 start=True, stop=True)
                ps_gh = psg2()
                nc.tensor.matmul(ps_gh[0:wv, 0:384], mT[:], wp[:, W_HH:W_HH + 384],
                                 start=True, stop=True)
                gi = sm_pool.tile([wv, 384], F32, name="gi", tag="gi")
                nc.vector.tensor_add(gi[:], ps_gi[0:wv, 0:384], bg[0:wv, BG_IH:BG_IH + 384])
                gh = sm_pool.tile([wv, 384], F32, name="gh", tag="gh")
                nc.vector.tensor_add(gh[:], ps_gh[0:wv, 0:384], bg[0:wv, BG_HH:BG_HH + 384])
                rz_pre = sm_pool.tile([wv, 256], F32, name="rz_pre", tag="rz_pre", bufs=1)
                nc.vector.tensor_add(rz_pre[:], gi[:, 0:256], gh[:, 0:256])
                rz_t = sm_pool.tile([wv, 256], F32, name="rz_t", tag="rz_t")
                nc.scalar.activation(rz_t[:], rz_pre[:], A.Tanh, scale=0.5)
                rz = sm_pool.tile([wv, 256], F32, name="rz", tag="rz")
                nc.vector.tensor_scalar(rz[:], rz_t[:], 0.5, 0.5, OP.mult, OP.add)
                n_pre = sm_pool.tile([wv, H], F32, name="n_pre", tag="n_pre", bufs=1)
                nc.vector.tensor_mul(n_pre[:], rz[:, 0:H], gh[:, 256:384])
                n_pre2 = sm_pool.tile([wv, H], F32, name="n_pre2", tag="n_pre2", bufs=1)
                nc.vector.tensor_add(n_pre2[:], n_pre[:], gi[:, 256:384])
                n_t = sm_pool.tile([wv, H], F32, name="n_t", tag="n_t")
                nc.scalar.activation(n_t[:], n_pre2[:], A.Tanh)
                dmn = sm_pool.tile([wv, H], F32, name="dmn", tag="dmn", bufs=1)
                nc.vector.tensor_sub(dmn[:], m_nm[:], n_t[:])
                zd = sm_pool.tile([wv, H], F32, name="zd", tag="zd", bufs=1)
                nc.vector.tensor_mul(zd[:], rz[:, H:256], dmn[:])
                m_nm = sm_pool.tile([wv, H], F32, name="m_nm", tag="m_nm")
                nc.vector.tensor_add(m_nm[:], n_t[:], zd[:])
                ps_mT = psg()
                nc.tensor.transpose(ps_mT[0:H, 0:wv], m_nm[:], ident[0:wv, 0:wv])
                mT = sm_pool.tile([H, wv], F32R, name="mT", tag="mT")
                nc.vector.tensor_copy(mT[:], ps_mT[0:H, 0:wv])

            # ----- head -----
            acf = sm_pool.tile([wv, 2 * H], F32R, name="acf", tag="acf")
            nc.vector.tensor_copy(acf[:, 0:H], xcf[:])
            nc.vector.tensor_copy(acf[:, H:2 * H], sf_w[:])
            for j in range(wv):
                g = gs + j
                pfr = st_pool.tile([1, H], F32R, name="pfr", tag="pfr")
                nc.sync.dma_start(out=pfr[:], in_=pfn[j:j + 1, :])
                acfr = st_pool.tile([1, 2 * H], F32R, name="acfr", tag="acfr")
                nc.sync.dma_start(out=acfr[:], in_=acf[j:j + 1, :])
                ps_o = ps_g.tile([H, 256], F32, name="psx", tag="psg2")
                nc.tensor.matmul(ps_o[:], pfr[:], acfr[:],
                                 start=True, stop=True)
                gk = st_pool.tile([H, 2 * H], F32, name="gk", tag="gk")
                act_lrelu(gk[:], ps_o[:])
                gkw = st_pool.tile([H, 2 * H], F32, name="gkw", tag="gkw")
                nc.vector.scalar_tensor_tensor(gkw[:], gk[:], 1.0, w2t[:],
                                               OP.mult, OP.mult,
                                               accum_out=partials[:, g:g + 1])

        # ---------- output ----------
        ps_fin = ps_g.tile([G, 256], F32, name="psx", tag="psg2")
        nc.tensor.matmul(ps_fin[0:G, 0:2], partials[:], ones_r[:], start=True, stop=True)
        ofin = gl_pool.tile([G, 1], F32, name="ofin", tag="ofin")
        nc.vector.tensor_scalar(ofin[:], ps_fin[0:G, 0:1], float(b_out_val), None, OP.add)
        nc.sync.dma_start(out=out_d[:], in_=ofin[:])

    if split:
        _split_waits(nc)
    return nc


def kernel(**inputs) -> np.ndarray:
    f = {k: np.asarray(v) for k, v in inputs.items()}
    f = {k: (v.astype(np.float32) if v.dtype != np.int32 else v) for k, v in f.items()}

    wblocks = [f['W_pc'], f['W_pp'], f['W_caff'], f['W_paff'], f['W_saff']]
    wblocks += [f['W_c2p'][i] for i in range(D)]
    wblocks += [f['W_hc0'][i] for i in range(D)]
    wblocks += [f['W_p2c'][i] for i in range(D)]
    wblocks += [f['W_hp0'][i] for i in range(D)]
    wblocks += [np.concatenate([f['W_mc1'][i], f['W_mp1'][i]], axis=1) for i in range(D)]
    wblocks += [f['W_ih'], f['W_hh']]
    wblocks += [f['W_hc1'][i] for i in range(D)]
    wblocks += [f['W_hp1'][i] for i in range(D)]
    wpack = np.ascontiguousarray(np.concatenate(wblocks, axis=1), dtype=np.float32)
    assert wpack.shape == (H, WCOLS), wpack.shape

    bcols = [f['b_pp'], f['b_paff'], f['b_pc'], f['b_caff']]
    bcols += [f['b_c2p'][i] for i in range(D)]
    bcols += [f['b_hc0'][i] for i in range(D)]
    bcols += [f['b_p2c'][i] for i in range(D)]
    bcols += [f['b_hp0'][i] for i in range(D)]
    bpack = np.stack(bcols, axis=1).astype(np.float32)
    assert bpack.shape == (H, BCOLS)

    bgparts = [np.tile(f['b_saff'][None, :], (G, 1))]
    bgparts += [np.tile(np.concatenate([f['b_mc1'][i], f['b_mp1'][i]])[None, :], (G, 1))
                for i in range(D)]
    bgparts += [np.tile(f['b_ih'][None, :], (G, 1)), np.tile(f['b_hh'][None, :], (G, 1))]
    bg16 = np.concatenate(bgparts, axis=1).astype(np.float32)
    assert bg16.shape == (G, BGCOLS)

    w2t = np.ascontiguousarray(f['W_out'].reshape(2 * H, H).T, dtype=np.float32)
    b_out_val = float(f['b_out'][0])

    key = ('nc', b_out_val)
    if key not in _CACHE:
        _CACHE[key] = _build(b_out_val)
    nc = _CACHE[key]

    comp = f['comp_feature'].reshape(NCORES, G, NC, H)
    prot = f['prot_feature'].reshape(NCORES, G, NP, H)
    gomp = f['gomp_feature'].reshape(NCORES, G, H)
    in_maps = []
    for c in range(NCORES):
        in_maps.append({
            "protT": np.ascontiguousarray(prot[c].transpose(2, 0, 1).reshape(H, G * NP)),
            "compT": np.ascontiguousarray(comp[c].transpose(2, 0, 1).reshape(H, G * NC)),
            "gompT": np.ascontiguousarray(gomp[c].T),
            "wpack": wpack, "bpack": bpack, "bg16": bg16, "w2t": w2t,
        })

    global LAST_EXEC_NS
    try:
        r = run_bass_kernel_spmd(nc, in_maps, list(range(NCORES)), trace=TRACE)
    except ModuleNotFoundError:
        r = run_bass_kernel_spmd(nc, in_maps, list(range(NCORES)))
    if getattr(r, "exec_time_ns", None):
        LAST_EXEC_NS = r.exec_time_ns
    res = r.results
    return np.concatenate([res[c]["out"] for c in range(NCORES)], axis=0)



# revision 20
# speedup vs baseline: 1.0221x; 1.0049x over previous
"""Trainium2 Bass kernel for AffinityNeuralNetworkMONN (gnn_message_passing).

Sharding: data-parallel over B=128 graphs -> 8 NeuronCores x 16 graphs.
Inside a core, graphs are processed in waves; per-graph heavy tensors use
a [H=128 partitions, nodes free] (T) layout so ACT bias/scale fuse per
partition; node-contraction operands are built NM via PE transposes.
All matmuls run as float32r (full-rate at N>=256). Softmax score rows are
reduced across partitions on GPSIMD and scattered to node-major via DMA.
Only tanh/exp/prelu ACT functions are used (one table set, no reloads).
"""
import sys
for p in ("/opt/trn_rl_repo", "/root/.axon_site/_ro/trn_rl_repo"):
    if p not in sys.path:
        sys.path.insert(0, p)

import numpy as np
import os
from contextlib import ExitStack

import concourse.bass as bass
import concourse.tile as tile
from concourse import mybir, masks
from concourse import bass_isa
from concourse.bass_utils import run_bass_kernel_spmd

F32 = mybir.dt.float32
F32R = mybir.dt.float32r
A = mybir.ActivationFunctionType
OP = mybir.AluOpType
AX = mybir.AxisListType

NCORES = 8
B, NC, NP, H, D = 128, 64, 1024, 128, 3
G = B // NCORES            # graphs per core = 16
WAVES = [2] * 8            # wave sizes (sum = 16); wv<=2 verified on HW
if os.environ.get("KWAVES"):
    WAVES = [int(x) for x in os.environ["KWAVES"].split(",")]
NCHUNK = NP // 128         # 8 p-chunks per graph

W_PC, W_PP, W_CAFF, W_PAFF, W_SAFF = 0, 128, 256, 384, 512
def W_C2P(i): return 640 + i * 128
def W_HC0(i): return 1024 + i * 128
def W_P2C(i): return 1408 + i * 128
def W_HP0(i): return 1792 + i * 128
def W_MCP(i): return 2176 + i * 256
W_IH, W_HH = 2944, 3328
def W_HC1(i): return 3712 + i
def W_HP1(i): return 3715 + i
WCOLS = 3718

B_PP, B_PAFF, B_PC, B_CAFF = 0, 1, 2, 3
def B_C2P(i): return 4 + i
def B_HC0(i): return 7 + i
def B_P2C(i): return 10 + i
def B_HP0(i): return 13 + i
BCOLS = 16

BG_SAFF = 0
def BG_MCP(i): return 128 + i * 256
BG_IH, BG_HH = 896, 1280
BGCOLS = 1664

_CACHE = {}
TRACE = False
LAST_EXEC_NS = None


def _split_waits(nc, keep=1):
    """walrus allows very few attached sync-waits per instruction (1 for the
    f32 self-loading matmul struct). Hoist excess waits into standalone
    EventSemaphore instructions right before the over-subscribed one."""
    for fn in nc.m.functions:
        for blk in fn.blocks:
            out = []
            for ins in blk.instructions:
                si = ins.sync_info
                if si is not None and si.on_wait and len(si.on_wait) > keep:
                    waits = list(si.on_wait)
                    for jj, w in enumerate(waits[:-keep]):
                        ev = mybir.InstNoOp(
                            name=f"{ins.name}-wsplit{jj}",
                            sync_info=mybir.SyncInfo(on_wait=[w], on_update=[]),
                            bass_nofuse=True)
                        ev.engine = ins.engine
                        out.append(ev)
                    si.on_wait = waits[-keep:]
                    ins.sync_info = si
                out.append(ins)
            blk.instructions = out


def _build(b_out_val: float, split: bool = True, sim_compat: bool = False):
    bisect = os.environ.get("KBISECT", "0") == "1"
    nc = bass.Bass()
    protT_d = nc.dram_tensor("protT", [H, G * NP], F32, kind="ExternalInput")
    compT_d = nc.dram_tensor("compT", [H, G * NC], F32, kind="ExternalInput")
    gompT_d = nc.dram_tensor("gompT", [H, G], F32, kind="ExternalInput")
    wpack_d = nc.dram_tensor("wpack", [H, WCOLS], F32, kind="ExternalInput")
    bpack_d = nc.dram_tensor("bpack", [H, BCOLS], F32, kind="ExternalInput")
    bg16_d = nc.dram_tensor("bg16", [G, BGCOLS], F32, kind="ExternalInput")
    w2t_d = nc.dram_tensor("w2t", [H, 2 * H], F32, kind="ExternalInput")
    out_d = nc.dram_tensor("out", [G, 1], F32, kind="ExternalOutput")

    with tile.TileContext(nc) as tc, ExitStack() as ctx:
        gl_pool = ctx.enter_context(tc.tile_pool(name="globals", bufs=1))
        per_pool = ctx.enter_context(tc.tile_pool(name="persist", bufs=1))
        st_pool = ctx.enter_context(tc.tile_pool(name="stream", bufs=2))
        sm_pool = ctx.enter_context(tc.tile_pool(name="small", bufs=2))
        ps_big = ctx.enter_context(tc.tile_pool(name="psBig", bufs=2, space="PSUM"))
        ps_t = ctx.enter_context(tc.tile_pool(name="psT", bufs=2, space="PSUM"))
        ps_g = ctx.enter_context(tc.tile_pool(name="psG", bufs=1, space="PSUM"))
        dr_pool = ctx.enter_context(tc.tile_pool(name="dram", bufs=2, space="DRAM"))

        def psg():
            return ps_g.tile([H, 512], F32, name="psg", tag="psg")

        def psg2():
            return ps_g.tile([H, 512], F32, name="psg2", tag="psg2")

        def act_lrelu(dst, src_ps, bias=0.0, accum_out=None):
            if not sim_compat:
                nc.scalar.activation(dst, src_ps, A.Prelu, bias=bias, alpha=0.1,
                                     accum_out=accum_out)
            else:
                shp = [dst.shape[0], int(np.prod(dst.shape[1:]))]
                t1 = sm_pool.tile(shp, F32, name="lr1", tag="lr_t1", bufs=1)
                t2 = sm_pool.tile(shp, F32, name="lr2", tag="lr_t2", bufs=1)
                nb = bias if isinstance(bias, float) else None
                nc.scalar.activation(t1[:], src_ps, A.Relu, bias=bias)
                if nb is None:
                    negb = sm_pool.tile([dst.shape[0], 1], F32, name="lrnb",
                                        tag="lr_nb", bufs=1)
                    nc.vector.tensor_scalar(negb[:], bias, -1.0, None, OP.mult)
                    nc.scalar.activation(t2[:], src_ps, A.Relu, scale=-1.0, bias=negb[:])
                else:
                    nc.scalar.activation(t2[:], src_ps, A.Relu, scale=-1.0, bias=-nb)
                nc.vector.scalar_tensor_tensor(dst, t2[:], -0.1, t1[:],
                                               OP.mult, OP.add, accum_out=accum_out)

        # ---------- preamble ----------
        wp = gl_pool.tile([H, WCOLS], F32R, name="wp", tag="wp")
        bp = gl_pool.tile([H, BCOLS], F32, name="bp", tag="bp")
        bg = gl_pool.tile([G, BGCOLS], F32, name="bg", tag="bg")
        w2t = gl_pool.tile([H, 2 * H], F32, name="w2t", tag="w2t")
        compT = gl_pool.tile([H, G * NC], F32R, name="compT", tag="compT")
        gompT = gl_pool.tile([H, G], F32R, name="gompT", tag="gompT")
        nc.sync.dma_start(out=wp[:], in_=wpack_d[:].bitcast(F32R))
        nc.sync.dma_start(out=bp[:], in_=bpack_d[:])
        nc.sync.dma_start(out=bg[:], in_=bg16_d[:])
        nc.sync.dma_start(out=w2t[:], in_=w2t_d[:])
        nc.sync.dma_start(out=compT[:], in_=compT_d[:].bitcast(F32R))
        nc.sync.dma_start(out=gompT[:], in_=gompT_d[:].bitcast(F32R))

        ident = gl_pool.tile([H, H], F32, name="ident", tag="ident")
        masks.make_identity(nc, ident[:])
        ones_r = gl_pool.tile([H, 2], F32R, name="ones_r", tag="ones_r")
        nc.vector.memset(ones_r[:].bitcast(F32), 1.0)
        identr = ident[:].bitcast(F32R)

        ceT = gl_pool.tile([H, G * NC], F32R, name="ceT", tag="ceT")
        pcT = gl_pool.tile([H, G * NC], F32R, name="pcT", tag="pcT")
        for (dst, wcol, bcol) in ((ceT, W_CAFF, B_CAFF), (pcT, W_PC, B_PC)):
            pscc = ps_big.tile([H, G * NC], F32, name="big", tag="big")
            nc.tensor.matmul(pscc[:, 0:512], wp[:, wcol:wcol + H], compT[:, 0:512],
                             start=True, stop=True)
            nc.tensor.matmul(pscc[:, 512:1024], wp[:, wcol:wcol + H], compT[:, 512:1024],
                             start=True, stop=True)
            act_lrelu(dst[:], pscc[:], bias=bp[:, bcol:bcol + 1])

        # CE_NM [128, 8, 128]: pair-transposed ce (abs graphs 2k, 2k+1 stacked)
        ce_nm = gl_pool.tile([H, 8, H], F32R, name="ce_nm", tag="ce_nm")
        for half in range(2):
            pst = ps_t.tile([H, 512], F32, name="pst", tag="pst")
            for k in range(4):
                pr = half * 4 + k
                nc.tensor.transpose(pst[:, k * 128:(k + 1) * 128],
                                    ceT[:, pr * 128:(pr + 1) * 128].bitcast(F32), ident[:])
            nc.vector.tensor_copy(ce_nm[:, half * 4:(half + 1) * 4, :],
                                  pst[:].rearrange("h (k c) -> h k c", k=4))

        cesum = gl_pool.tile([H, G], F32, name="cesum", tag="cesum")
        nc.vector.tensor_reduce(cesum[:],
                                ceT[:].bitcast(F32).rearrange("h (g c) -> h g c", g=G),
                                AX.X, OP.add)
        peacc = gl_pool.tile([H, G], F32, name="peacc", tag="peacc")
        partials = gl_pool.tile([H, G], F32R, name="partials", tag="partials")

        # ---------- waves ----------
        g0 = 0
        for wv in WAVES:
            gs, ge = g0, g0 + wv
            g0 = ge

            peT = [per_pool.tile([H, NP], F32R, name=f"peT{j}", tag=f"peT{j}") for j in range(wv)]
            pairslab = [per_pool.tile([H, NP], F32R, name=f"pairs{q}", tag=f"pairs{q}")
                        for q in range((wv + 1) // 2)]
            pair = [pairslab[j // 2][(j % 2) * 64:(j % 2) * 64 + 64, :] for j in range(wv)]
            pwe = [per_pool.tile([H, NCHUNK, 65], F32R, name=f"pwe{j}", tag=f"pwe{j}") for j in range(wv)]
            ppe = [per_pool.tile([H, NCHUNK, 256], F32R, name=f"ppe{j}", tag=f"ppe{j}") for j in range(wv)]

            # ----- phase A -----
            for j in range(wv):
                g = gs + j
                protT = st_pool.tile([H, NP], F32R, name="protT", tag="protT")
                nc.sync.dma_start(out=protT[:],
                                  in_=protT_d[:, g * NP:(g + 1) * NP].bitcast(F32R))

                ps_pp = ps_big.tile([H, NP], F32, name="big", tag="big")
                nc.tensor.matmul(ps_pp[:, 0:512], wp[:, W_PP:W_PP + H], protT[:, 0:512],
                                 start=True, stop=True)
                nc.tensor.matmul(ps_pp[:, 512:1024], wp[:, W_PP:W_PP + H],
                                 protT[:, 512:1024], start=True, stop=True)
                ppT = st_pool.tile([H, NP], F32R, name="ppT", tag="ppT")
                act_lrelu(ppT[:], ps_pp[:], bias=bp[:, B_PP:B_PP + 1])

                ps_pe = ps_big.tile([H, NP], F32, name="big", tag="big")
                nc.tensor.matmul(ps_pe[:, 0:512], wp[:, W_PAFF:W_PAFF + H], protT[:, 0:512],
                                 start=True, stop=True)
                nc.tensor.matmul(ps_pe[:, 512:1024], wp[:, W_PAFF:W_PAFF + H],
                                 protT[:, 512:1024], start=True, stop=True)
                act_lrelu(peT[j][:], ps_pe[:], bias=bp[:, B_PAFF:B_PAFF + 1],
                          accum_out=peacc[:, g:g + 1])

                # pairwise = sigmoid(pc @ pp^T) = 0.5 + 0.5*tanh(z/2)
                hb = (j % 2) * 64
                ps_pw = ps_big.tile([H, NP], F32, name="big", tag="big")
                nc.tensor.matmul(ps_pw[0:64, 0:512], pcT[:, g * NC:(g + 1) * NC],
                                 ppT[:, 0:512], start=True, stop=True)
                nc.tensor.matmul(ps_pw[0:64, 512:1024], pcT[:, g * NC:(g + 1) * NC],
                                 ppT[:, 512:1024], start=True, stop=True)
                pw_t = st_pool.tile([H, NP], F32, name="pw_t", tag="pw_t")
                nc.scalar.activation(pw_t[0:64, :], ps_pw[0:64, :], A.Tanh, scale=0.5)
                if hb == 0:
                    nc.vector.tensor_scalar(pair[j], pw_t[0:64, :], 0.5, 0.5,
                                            OP.mult, OP.add)
                else:
                    pair_st = st_pool.tile([64, NP], F32, name="pair_st", tag="pair_st")
                    nc.vector.tensor_scalar(pair_st[:], pw_t[0:64, :], 0.5, 0.5,
                                            OP.mult, OP.add)
                    nc.sync.dma_start(out=pair[j].bitcast(F32), in_=pair_st[:])

                # pairwiseT -> pwe cols 0:64
                for half in range(2):
                    pstp = ps_t.tile([H, 512], F32, name="pst", tag="pst")
                    for k in range(4):
                        ch = half * 4 + k
                        nc.tensor.transpose(pstp[:, k * 128:k * 128 + 64],
                                            pair[j][:, ch * 128:(ch + 1) * 128].bitcast(F32),
                                            ident[hb:hb + 64, hb:hb + 64])
                    nc.vector.tensor_copy(
                        pwe[j][:, half * 4:(half + 1) * 4, 0:64],
                        pstp[:].rearrange("h (k c) -> h k c", k=4)[:, :, 0:64])

                # peT transposes -> ppe cols 128:256 (pe_NM)
                for half in range(2):
                    psq = ps_t.tile([H, 512], F32, name="pst", tag="pst")
                    for k in range(4):
                        ch = half * 4 + k
                        nc.tensor.transpose(psq[:, k * 128:(k + 1) * 128],
                                            peT[j][:, ch * 128:(ch + 1) * 128].bitcast(F32),
                                            ident[:])
                    nc.vector.tensor_copy(
                        ppe[j][:, half * 4:(half + 1) * 4, 128:256],
                        psq[:].rearrange("h (k c) -> h k c", k=4))

            # sf for this wave: lrelu(gomp @ W_saff + b_saff)
            ps_sf = psg()
            nc.tensor.matmul(ps_sf[0:wv, 0:256], gompT[:, gs:ge],
                             wp[:, W_SAFF:W_SAFF + 256], start=True, stop=True)
            sf_pre = sm_pool.tile([wv, H], F32, name="sf_pre", tag="sf_pre", bufs=1)
            nc.vector.tensor_add(sf_pre[:], ps_sf[0:wv, 0:H], bg[0:wv, BG_SAFF:BG_SAFF + H])
            sf_w = sm_pool.tile([wv, H], F32, name="sf_w", tag="sf_w")
            act_lrelu(sf_w[:], sf_pre[:])

            # m0
            mT = sm_pool.tile([H, wv], F32R, name="mT", tag="mT")
            nc.vector.scalar_tensor_tensor(mT[:], cesum[:, gs:ge], 1.0 / (NC * NP),
                                           peacc[:, gs:ge], OP.mult, OP.mult)
            ps_m0 = psg()
            nc.tensor.transpose(ps_m0[0:wv, 0:H], mT[:].bitcast(F32), ident[:])
            m_nm = sm_pool.tile([wv, H], F32, name="m_nm", tag="m_nm")
            nc.vector.tensor_copy(m_nm[:], ps_m0[0:wv, 0:H])

            xcf = sm_pool.tile([wv, H], F32, name="xcf", tag="xcf")
            pfn = sm_pool.tile([wv, H], F32R, name="pfn", tag="pfn")

            # ----- phase B: D iterations -----
            for i in range(D):
                csl = slice(gs * NC, ge * NC)
                ps_cp = ps_big.tile([H, wv * NC], F32, name="big", tag="big")
                nc.tensor.matmul(ps_cp[:], wp[:, W_C2P(i):W_C2P(i) + H], ceT[:, csl],
                                 start=True, stop=True)
                cpreT = sm_pool.tile([H, wv * NC], F32, name="cpreT", tag="cpreT")
                nc.scalar.activation(cpreT[:], ps_cp[:], A.Tanh,
                                     bias=bp[:, B_C2P(i):B_C2P(i) + 1])
                ps_h0 = ps_big.tile([H, wv * NC], F32, name="big", tag="big")
                nc.tensor.matmul(ps_h0[:], wp[:, W_HC0(i):W_HC0(i) + H], ceT[:, csl],
                                 start=True, stop=True)
                hc0T = sm_pool.tile([H, wv * NC], F32, name="hc0T", tag="hc0T")
                nc.scalar.activation(hc0T[:], ps_h0[:], A.Tanh,
                                     bias=bp[:, B_HC0(i):B_HC0(i) + 1])

                # c_pre_NM (graph-pair transposes; wave starts are odd-free:
                # waves [5,5,3] start at 0,5,10 -> pairs may straddle; use
                # per-graph 64-col transposes into fixed parity slots)
                cpre_nm = sm_pool.tile([H, 2, H], F32R, name="cpre_nm", tag="cpre_nm")
                psq2 = ps_t.tile([H, 512], F32, name="pst", tag="pst")
                for j in range(wv):
                    nc.tensor.transpose(psq2[0:64, j * 128:(j + 1) * 128],
                                        cpreT[:, j * 64:(j + 1) * 64], ident[:])
                # evens -> partitions 0:64 (DVE), odds -> 64:128 (DMA shifts partitions)
                ne, no = (wv + 1) // 2, wv // 2
                psq2v = psq2[0:64, 0:wv * 128].rearrange("c (j h) -> c j h", j=wv)
                nc.vector.tensor_copy(cpre_nm[0:64, 0:ne, :], psq2v[:, 0::2, :])
                cpre_odd = sm_pool.tile([64, 2, H], F32, name="cpre_odd", tag="cpre_odd", bufs=1)
                nc.vector.tensor_copy(cpre_odd[:, 0:no, :], psq2v[:, 1::2, :])
                nc.sync.dma_start(out=cpre_nm[64:128, 0:no, :].bitcast(F32),
                                  in_=cpre_odd[:, 0:no, :])

                # mc1/mp1 batched
                ps_mm = psg()
                nc.tensor.matmul(ps_mm[0:wv, 0:256], mT[:], wp[:, W_MCP(i):W_MCP(i) + 256],
                                 start=True, stop=True)
                mcp_pre = sm_pool.tile([wv, 256], F32, name="mcp_pre", tag="mcp_pre", bufs=1)
                nc.vector.tensor_add(mcp_pre[:], ps_mm[0:wv, 0:256],
                                     bg[0:wv, BG_MCP(i):BG_MCP(i) + 256])
                mcp = sm_pool.tile([wv, 256], F32, name="mcp", tag="mcp")
                nc.scalar.activation(mcp[:], mcp_pre[:], A.Tanh)
                ps_mt = psg()
                nc.tensor.transpose(ps_mt[0:H, 0:wv], mcp[:, 0:H], ident[0:wv, 0:wv])
                mc1T = sm_pool.tile([H, wv], F32, name="mc1T", tag="mc1T")
                nc.vector.tensor_copy(mc1T[:], ps_mt[0:H, 0:wv])
                ps_mt2 = psg()
                nc.tensor.transpose(ps_mt2[0:H, 0:wv], mcp[:, H:256], ident[0:wv, 0:wv])
                mp1T = sm_pool.tile([H, wv], F32, name="mp1T", tag="mp1T")
                nc.vector.tensor_copy(mp1T[:], ps_mt2[0:H, 0:wv])

                wc_w = sm_pool.tile([H, wv], F32, name="wc_w", tag="wc_w")
                nc.vector.tensor_scalar(wc_w[:], mc1T[:],
                                        wp[:, W_HC1(i):W_HC1(i) + 1].bitcast(F32),
                                        None, OP.mult)
                wp_w = sm_pool.tile([H, wv], F32, name="wp_w", tag="wp_w")
                nc.vector.tensor_scalar(wp_w[:], mp1T[:],
                                        wp[:, W_HP1(i):W_HP1(i) + 1].bitcast(F32),
                                        None, OP.mult)

                qcw = sm_pool.tile([H, wv * NC], F32R, name="qcw", tag="qcw")
                esum = sm_pool.tile([1, wv], F32, name="esum", tag="esum")
                pfu = sm_pool.tile([wv, H], F32, name="pfu", tag="pfu")
                pf_stage = sm_pool.tile([H, wv * H], F32, name="pf_stage", tag="pf_stage", bufs=1)

                # ----- per graph heavy chain (pass 1: through e-scatter) -----
                for j in range(wv):
                    g = gs + j
                    ps_p1 = ps_big.tile([H, NP], F32, name="big", tag="big")
                    nc.tensor.matmul(ps_p1[:, 0:512], wp[:, W_P2C(i):W_P2C(i) + H],
                                     peT[j][:, 0:512], start=True, stop=True)
                    nc.tensor.matmul(ps_p1[:, 512:1024], wp[:, W_P2C(i):W_P2C(i) + H],
                                     peT[j][:, 512:1024], start=True, stop=True)
                    ppreT = st_pool.tile([H, NP], F32, name="ppreT", tag="ppreT")
                    nc.scalar.activation(ppreT[:], ps_p1[:], A.Tanh,
                                         bias=bp[:, B_P2C(i):B_P2C(i) + 1])
                    ps_p2 = ps_big.tile([H, NP], F32, name="big", tag="big")
                    nc.tensor.matmul(ps_p2[:, 0:512], wp[:, W_HP0(i):W_HP0(i) + H],
                                     peT[j][:, 0:512], start=True, stop=True)
                    nc.tensor.matmul(ps_p2[:, 512:1024], wp[:, W_HP0(i):W_HP0(i) + H],
                                     peT[j][:, 512:1024], start=True, stop=True)
                    hp0T = st_pool.tile([H, NP], F32, name="hp0T", tag="hp0T")
                    nc.scalar.activation(hp0T[:], ps_p2[:], A.Tanh,
                                         bias=bp[:, B_HP0(i):B_HP0(i) + 1])

                    for half in range(2):
                        psq3 = ps_t.tile([H, 512], F32, name="pst", tag="pst")
                        for k in range(4):
                            ch = half * 4 + k
                            nc.tensor.transpose(psq3[:, k * 128:(k + 1) * 128],
                                                ppreT[:, ch * 128:(ch + 1) * 128], ident[:])
                        nc.vector.tensor_copy(
                            ppe[j][:, half * 4:(half + 1) * 4, 0:128],
                            psq3[:].rearrange("h (k c) -> h k c", k=4))

                    ps_cp2 = ps_big.tile([H, NP], F32, name="big", tag="big")
                    qb = (j % 2) * 64
                    lhs_cp = cpre_nm[qb:qb + 64, j // 2, :]
                    nc.tensor.matmul(ps_cp2[:, 0:512], lhs_cp, pair[j][:, 0:512],
                                     start=True, stop=True)
                    nc.tensor.matmul(ps_cp2[:, 512:1024], lhs_cp, pair[j][:, 512:1024],
                                     start=True, stop=True)

                    qwT = st_pool.tile([H, NP], F32R, name="qwT", tag="qwT")
                    nc.vector.scalar_tensor_tensor(qwT[:], ps_cp2[:], wp_w[:, j:j + 1],
                                                   hp0T[:], OP.mult, OP.mult)
                    ps_s = ps_big.tile([H, NP], F32, name="big", tag="big")
                    nc.tensor.matmul(ps_s[0:1, 0:512], ones_r[:, 0:1], qwT[:, 0:512],
                                     start=True, stop=True)
                    nc.tensor.matmul(ps_s[0:1, 512:1024], ones_r[:, 0:1], qwT[:, 512:1024],
                                     start=True, stop=True)
                    e_row = st_pool.tile([1, NP], F32, name="e_row", tag="e_row")
                    nc.scalar.activation(e_row[:], ps_s[0:1, :], A.Exp,
                                         accum_out=esum[0:1, j:j + 1])
                    s_dr = dr_pool.tile([NP], F32, name="s_dr", tag="s_dr")
                    nc.sync.dma_start(out=s_dr[:], in_=e_row[:])
                    if bisect:
                        nc.sync.dma_start(out=pwe[j][:, :, 64].bitcast(F32),
                                          in_=s_dr[:].rearrange("(p c) -> p c", c=NCHUNK))
                    else:
                        nc.sync.dma_start(out=pwe[j][:, :, 64].bitcast(F32),
                                          in_=s_dr[:].rearrange("(c p) -> p c", p=128))

                # ----- pass 2: node-contraction per graph -----
                for j in range(wv):
                    g = gs + j
                    ps_cc = ps_g.tile([65, 256], F32, name="psx", tag="psg2")
                    for k in range(NCHUNK):
                        nc.tensor.matmul(ps_cc[:], pwe[j][:, k, :], ppe[j][:, k, :],
                                         start=(k == 0), stop=(k == NCHUNK - 1))
                    p2c = st_pool.tile([64, H], F32, name="p2c", tag="p2c")
                    nc.vector.tensor_copy(p2c[:], ps_cc[0:64, 0:128])
                    ps_tc = ps_t.tile([H, 512], F32, name="pst", tag="pst")
                    nc.tensor.transpose(ps_tc[:, 0:64], p2c[:], ident[0:64, 0:64])
                    nc.vector.scalar_tensor_tensor(qcw[:, j * NC:(j + 1) * NC],
                                                   ps_tc[:, 0:64], wc_w[:, j:j + 1],
                                                   hc0T[:, j * NC:(j + 1) * NC],
                                                   OP.mult, OP.mult)
                    nc.vector.tensor_copy(pf_stage[64:65, j * H:(j + 1) * H],
                                          ps_cc[64:65, 128:256])

                pf_dr = dr_pool.tile([G * H], F32, name="pf_dr", tag="pf_dr")
                nc.sync.dma_start(out=pf_dr[0:wv * H], in_=pf_stage[64:65, :])
                nc.sync.dma_start(out=pfu[:],
                                  in_=pf_dr[0:wv * H].rearrange("(j h) -> j h", j=wv))
                es_dr = dr_pool.tile([G], F32, name="es_dr", tag="es_dr")
                nc.sync.dma_start(out=es_dr[0:wv], in_=esum[:])
                esum_nm = sm_pool.tile([wv, 1], F32, name="esum_nm", tag="esum_nm")
                nc.sync.dma_start(out=esum_nm[:], in_=es_dr[0:wv].rearrange("(j a) -> j a", j=wv))
                esum2 = sm_pool.tile([wv, 1], F32, name="esum2", tag="esum2")
                nc.vector.tensor_scalar(esum2[:], esum_nm[:], 1e-6, None, OP.add)
                rec_p = sm_pool.tile([wv, 1], F32, name="rec_p", tag="rec_p")
                nc.vector.reciprocal(rec_p[:], esum2[:])
                nc.vector.tensor_scalar(pfn[:], pfu[:], rec_p[:], None, OP.mult)

                # ----- batched c softmax + cf + pf + GRU -----
                ps_sc = psg2()
                nc.tensor.matmul(ps_sc[0:1, 0:wv * NC], ones_r[:, 0:1], qcw[:],
                                 start=True, stop=True)
                sc_rowt = sm_pool.tile([1, wv * NC], F32, name="sc_rowt", tag="sc_rowt",
                                       bufs=1)
                nc.scalar.activation(sc_rowt[:], ps_sc[0:1, 0:wv * NC], A.Copy)
                sc_dr = dr_pool.tile([G * NC], F32, name="sc_dr", tag="sc_dr")
                nc.sync.dma_start(out=sc_dr[0:wv * NC], in_=sc_rowt[:])
                sc_nm = sm_pool.tile([wv, NC], F32, name="sc_nm", tag="sc_nm")
                nc.sync.dma_start(out=sc_nm[:],
                                  in_=sc_dr[0:wv * NC].rearrange("(g c) -> g c", g=wv))  # contiguous
                negmax = sm_pool.tile([wv, 1], F32, name="negmax", tag="negmax")
                nc.vector.tensor_reduce(negmax[:], sc_nm[:], AX.X, OP.max, negate=True)
                eac = sm_pool.tile([wv, NC], F32, name="eac", tag="eac")
                sumec = sm_pool.tile([wv, 1], F32, name="sumec", tag="sumec")
                nc.scalar.activation(eac[:], sc_nm[:], A.Exp, bias=negmax[:],
                                     accum_out=sumec[:])
                rec_c = sm_pool.tile([wv, 1], F32, name="rec_c", tag="rec_c")
                nc.vector.reciprocal(rec_c[:], sumec[:])
                ac_nm = sm_pool.tile([wv, NC], F32, name="ac_nm", tag="ac_nm")
                nc.vector.tensor_scalar(ac_nm[:], eac[:], rec_c[:], None, OP.mult)
                # transpose into both parity halves
                ps_at = psg()
                nc.tensor.transpose(ps_at[0:NC, 0:wv], ac_nm[:], ident[0:wv, 0:wv])
                ac_stage = sm_pool.tile([NC, wv], F32, name="ac_stage", tag="ac_stage")
                nc.vector.tensor_copy(ac_stage[:], ps_at[0:NC, 0:wv])
                acT2 = sm_pool.tile([H, wv], F32R, name="acT2", tag="acT2")
                nc.vector.tensor_copy(acT2[0:NC, :], ac_stage[:])
                nc.sync.dma_start(out=acT2[64:128, :].bitcast(F32), in_=ac_stage[:])

                ps_cf = psg2()
                for j in range(wv):
                    g = gs + j
                    hb = (g % 2) * 64
                    nc.tensor.matmul(ps_cf[0:1, j * H:(j + 1) * H],
                                     acT2[hb:hb + 64, j:j + 1],
                                     ce_nm[hb:hb + 64, g // 2, :], start=True, stop=True)
                cf_row = sm_pool.tile([1, wv * H], F32, name="cf_row", tag="cf_row", bufs=1)
                nc.vector.tensor_copy(cf_row[:], ps_cf[0:1, 0:wv * H])
                cf_dr = dr_pool.tile([G * H], F32, name="cf_dr", tag="cf_dr")
                nc.sync.dma_start(out=cf_dr[0:wv * H], in_=cf_row[:])
                nc.sync.dma_start(out=xcf[:],
                                  in_=cf_dr[0:wv * H].rearrange("(j h) -> j h", j=wv))  # contiguous

                if i == D - 1:
                    continue
                xw = sm_pool.tile([wv, H], F32, name="xw", tag="xw")
                nc.vector.tensor_mul(xw[:], xcf[:], pfn[:].bitcast(F32))
                ps_xt = psg()
                nc.tensor.transpose(ps_xt[0:H, 0:wv], xw[:], ident[0:wv, 0:wv])
                xT = sm_pool.tile([H, wv], F32R, name="xT", tag="xT")
                nc.vector.tensor_copy(xT[:], ps_xt[0:H, 0:wv])
                ps_gi = psg()
                nc.tensor.matmul(ps_gi[0:wv, 0:384], xT[:], wp[:, W_IH:W_IH + 384],
                                 start=True, stop=True)
                ps_gh = psg2()
                nc.tensor.matmul(ps_gh[0:wv, 0:384], mT[:], wp[:, W_HH:W_HH + 384],
                                 start=True, stop=True)
                gi = sm_pool.tile([wv, 384], F32, name="gi", tag="gi")
                nc.vector.tensor_add(gi[:], ps_gi[0:wv, 0:384], bg[0:wv, BG_IH:BG_IH + 384])
                gh = sm_pool.tile([wv, 384], F32, name="gh", tag="gh")
                nc.vector.tensor_add(gh[:], ps_gh[0:wv, 0:384], bg[0:wv, BG_HH:BG_HH + 384])
                rz_pre = sm_pool.tile([wv, 256], F32, name="rz_pre", tag="rz_pre", bufs=1)
                nc.vector.tensor_add(rz_pre[:], gi[:, 0:256], gh[:, 0:256])
                rz_t = sm_pool.tile([wv, 256], F32, name="rz_t", tag="rz_t")
                nc.scalar.activation(rz_t[:], rz_pre[:], A.Tanh, scale=0.5)
                rz = sm_pool.tile([wv, 256], F32, name="rz", tag="rz")
                nc.vector.tensor_scalar(rz[:], rz_t[:], 0.5, 0.5, OP.mult, OP.add)
                n_pre = sm_pool.tile([wv, H], F32, name="n_pre", tag="n_pre", bufs=1)
                nc.vector.tensor_mul(n_pre[:], rz[:, 0:H], gh[:, 256:384])
                n_pre2 = sm_pool.tile([wv, H], F32, name="n_pre2", tag="n_pre2", bufs=1)
                nc.vector.tensor_add(n_pre2[:], n_pre[:], gi[:, 256:384])
                n_t = sm_pool.tile([wv, H], F32, name="n_t", tag="n_t")
                nc.scalar.activation(n_t[:], n_pre2[:], A.Tanh)
                dmn = sm_pool.tile([wv, H], F32, name="dmn", tag="dmn", bufs=1)
                nc.vector.tensor_sub(dmn[:], m_nm[:], n_t[:])
                zd = sm_pool.tile([wv, H], F32, name="zd", tag="zd", bufs=1)
                nc.vector.tensor_mul(zd[:], rz[:, H:256], dmn[:])
                m_nm = sm_pool.tile([wv, H], F32, name="m_nm", tag="m_nm")
                nc.vector.tensor_add(m_nm[:], n_t[:], zd[:])
                ps_mT = psg()
                nc.tensor.transpose(ps_mT[0:H, 0:wv], m_nm[:], ident[0:wv, 0:wv])
                mT = sm_pool.tile([H, wv], F32R, name="mT", tag="mT")
                nc.vector.tensor_copy(mT[:], ps_mT[0:H, 0:wv])

            # ----- head -----
            acf = sm_pool.tile([wv, 2 * H], F32R, name="acf", tag="acf")
            nc.vector.tensor_copy(acf[:, 0:H], xcf[:])
            nc.vector.tensor_copy(acf[:, H:2 * H], sf_w[:])
            for j in range(wv):
                g = gs + j
                pfr = st_pool.tile([1, H], F32R, name="pfr", tag="pfr")
                nc.sync.dma_start(out=pfr[:], in_=pfn[j:j + 1, :])
                acfr = st_pool.tile([1, 2 * H], F32R, name="acfr", tag="acfr")
                nc.sync.dma_start(out=acfr[:], in_=acf[j:j + 1, :])
                ps_o = ps_g.tile([H, 256], F32, name="psx", tag="psg2")
                nc.tensor.matmul(ps_o[:], pfr[:], acfr[:],
                                 start=True, stop=True)
                gk = st_pool.tile([H, 2 * H], F32, name="gk", tag="gk")
                act_lrelu(gk[:], ps_o[:])
                gkw = st_pool.tile([H, 2 * H], F32, name="gkw", tag="gkw")
                nc.vector.scalar_tensor_tensor(gkw[:], gk[:], 1.0, w2t[:],
                                               OP.mult, OP.mult,
                                               accum_out=partials[:, g:g + 1])

        # ---------- output ----------
        ps_fin = ps_g.tile([G, 256], F32, name="psx", tag="psg2")
        nc.tensor.matmul(ps_fin[0:G, 0:2], partials[:], ones_r[:], start=True, stop=True)
        ofin = gl_pool.tile([G, 1], F32, name="ofin", tag="ofin")
        nc.vector.tensor_scalar(ofin[:], ps_fin[0:G, 0:1], float(b_out_val), None, OP.add)
        nc.sync.dma_start(out=out_d[:], in_=ofin[:])

    if split:
        _split_waits(nc)
    return nc


def kernel(**inputs) -> np.ndarray:
    f = {k: np.asarray(v) for k, v in inputs.items()}
    f = {k: (v.astype(np.float32) if v.dtype != np.int32 else v) for k, v in f.items()}

    wblocks = [f['W_pc'], f['W_pp'], f['W_caff'], f['W_paff'], f['W_saff']]
    wblocks += [f['W_c2p'][i] for i in range(D)]
    wblocks += [f['W_hc0'][i] for i in range(D)]
    wblocks += [f['W_p2c'][i] for i in range(D)]
    wblocks += [f['W_hp0'][i] for i in range(D)]
    wblocks += [np.concatenate([f['W_mc1'][i], f['W_mp1'][i]], axis=1) for i in range(D)]
    wblocks += [f['W_ih'], f['W_hh']]
    wblocks += [f['W_hc1'][i] for i in range(D)]
    wblocks += [f['W_hp1'][i] for i in range(D)]
    wpack = np.ascontiguousarray(np.concatenate(wblocks, axis=1), dtype=np.float32)
    assert wpack.shape == (H, WCOLS), wpack.shape

    bcols = [f['b_pp'], f['b_paff'], f['b_pc'], f['b_caff']]
    bcols += [f['b_c2p'][i] for i in range(D)]
    bcols += [f['b_hc0'][i] for i in range(D)]
    bcols += [f['b_p2c'][i] for i in range(D)]
    bcols += [f['b_hp0'][i] for i in range(D)]
    bpack = np.stack(bcols, axis=1).astype(np.float32)
    assert bpack.shape == (H, BCOLS)

    bgparts = [np.tile(f['b_saff'][None, :], (G, 1))]
    bgparts += [np.tile(np.concatenate([f['b_mc1'][i], f['b_mp1'][i]])[None, :], (G, 1))
                for i in range(D)]
    bgparts += [np.tile(f['b_ih'][None, :], (G, 1)), np.tile(f['b_hh'][None, :], (G, 1))]
    bg16 = np.concatenate(bgparts, axis=1).astype(np.float32)
    assert bg16.shape == (G, BGCOLS)

    w2t = np.ascontiguousarray(f['W_out'].reshape(2 * H, H).T, dtype=np.float32)
    b_out_val = float(f['b_out'][0])

    key = ('nc', b_out_val)
    if key not in _CACHE:
        _CACHE[key] = _build(b_out_val)
    nc = _CACHE[key]

    comp = f['comp_feature'].reshape(NCORES, G, NC, H)
    prot = f['prot_feature'].reshape(NCORES, G, NP, H)
    gomp = f['gomp_feature'].reshape(NCORES, G, H)
    in_maps = []
    for c in range(NCORES):
        in_maps.append({
            "protT": np.ascontiguousarray(prot[c].transpose(2, 0, 1).reshape(H, G * NP)),
            "compT": np.ascontiguousarray(comp[c].transpose(2, 0, 1).reshape(H, G * NC)),
            "gompT": np.ascontiguousarray(gomp[c].T),
            "wpack": wpack, "bpack": bpack, "bg16": bg16, "w2t": w2t,
        })

    global LAST_EXEC_NS
    try:
        r = run_bass_kernel_spmd(nc, in_maps, list(range(NCORES)), trace=TRACE)
    except ModuleNotFoundError:
        r = run_bass_kernel_spmd(nc, in_maps, list(range(NCORES)))
    if getattr(r, "exec_time_ns", None):
        LAST_EXEC_NS = r.exec_time_ns
    res = r.results
    return np.concatenate([res[c]["out"] for c in range(NCORES)], axis=0)



# revision 21
# speedup vs baseline: 1.1369x; 1.1123x over previous
"""Trainium2 Bass kernel for AffinityNeuralNetworkMONN (gnn_message_passing).

Sharding: data-parallel over B=128 graphs -> 8 NeuronCores x 16 graphs.
Inside a core, graphs are processed in waves; per-graph heavy tensors use
a [H=128 partitions, nodes free] (T) layout so ACT bias/scale fuse per
partition; node-contraction operands are built NM via PE transposes.
All matmuls run as float32r (full-rate at N>=256). Softmax score rows are
reduced across partitions on GPSIMD and scattered to node-major via DMA.
Only tanh/exp/prelu ACT functions are used (one table set, no reloads).
"""
import sys
for p in ("/opt/trn_rl_repo", "/root/.axon_site/_ro/trn_rl_repo"):
    if p not in sys.path:
        sys.path.insert(0, p)

import numpy as np
import os
from contextlib import ExitStack

import concourse.bass as bass
import concourse.tile as tile
from concourse import mybir, masks
from concourse import bass_isa
from concourse.bass_utils import run_bass_kernel_spmd

F32 = mybir.dt.float32
F32R = mybir.dt.float32r
A = mybir.ActivationFunctionType
OP = mybir.AluOpType
AX = mybir.AxisListType

NCORES = 8
B, NC, NP, H, D = 128, 64, 1024, 128, 3
G = B // NCORES            # graphs per core = 16
WAVES = [2] * 8            # wave sizes (sum = 16); wv<=2 verified on HW
if os.environ.get("KWAVES"):
    WAVES = [int(x) for x in os.environ["KWAVES"].split(",")]
NCHUNK = NP // 128         # 8 p-chunks per graph

W_PC, W_PP, W_CAFF, W_PAFF, W_SAFF = 0, 128, 256, 384, 512
def W_C2P(i): return 640 + i * 128
def W_HC0(i): return 1024 + i * 128
def W_P2C(i): return 1408 + i * 128
def W_HP0(i): return 1792 + i * 128
def W_MCP(i): return 2176 + i * 256
W_IH, W_HH = 2944, 3328
def W_HC1(i): return 3712 + i
def W_HP1(i): return 3715 + i
WCOLS = 3718

B_PP, B_PAFF, B_PC, B_CAFF = 0, 1, 2, 3
def B_C2P(i): return 4 + i
def B_HC0(i): return 7 + i
def B_P2C(i): return 10 + i
def B_HP0(i): return 13 + i
BCOLS = 16

BG_SAFF = 0
def BG_MCP(i): return 128 + i * 256
BG_IH, BG_HH = 896, 1280
BGCOLS = 1664

_CACHE = {}
TRACE = False
LAST_EXEC_NS = None


def _split_waits(nc, keep=1):
    """walrus allows very few attached sync-waits per instruction (1 for the
    f32 self-loading matmul struct). Hoist excess waits into standalone
    EventSemaphore instructions right before the over-subscribed one."""
    for fn in nc.m.functions:
        for blk in fn.blocks:
            out = []
            for ins in blk.instructions:
                si = ins.sync_info
                if si is not None and si.on_wait and len(si.on_wait) > keep:
                    waits = list(si.on_wait)
                    for jj, w in enumerate(waits[:-keep]):
                        ev = mybir.InstNoOp(
                            name=f"{ins.name}-wsplit{jj}",
                            sync_info=mybir.SyncInfo(on_wait=[w], on_update=[]),
                            bass_nofuse=True)
                        ev.engine = ins.engine
                        out.append(ev)
                    si.on_wait = waits[-keep:]
                    ins.sync_info = si
                out.append(ins)
            blk.instructions = out


def _build(b_out_val: float, split: bool = True, sim_compat: bool = False):
    bisect = os.environ.get("KBISECT", "0") == "1"
    nc = bass.Bass()
    protT_d = nc.dram_tensor("protT", [H, G * NP], F32, kind="ExternalInput")
    compT_d = nc.dram_tensor("compT", [H, G * NC], F32, kind="ExternalInput")
    gompT_d = nc.dram_tensor("gompT", [H, G], F32, kind="ExternalInput")
    wpack_d = nc.dram_tensor("wpack", [H, WCOLS], F32, kind="ExternalInput")
    bpack_d = nc.dram_tensor("bpack", [H, BCOLS], F32, kind="ExternalInput")
    bg16_d = nc.dram_tensor("bg16", [G, BGCOLS], F32, kind="ExternalInput")
    w2t_d = nc.dram_tensor("w2t", [H, 2 * H], F32, kind="ExternalInput")
    out_d = nc.dram_tensor("out", [G, 1], F32, kind="ExternalOutput")

    with tile.TileContext(nc) as tc, ExitStack() as ctx:
        gl_pool = ctx.enter_context(tc.tile_pool(name="globals", bufs=1))
        per_pool = ctx.enter_context(tc.tile_pool(name="persist", bufs=1))
        st_pool = ctx.enter_context(tc.tile_pool(name="stream", bufs=2))
        sm_pool = ctx.enter_context(tc.tile_pool(name="small", bufs=2))
        ps_big = ctx.enter_context(tc.tile_pool(name="psBig", bufs=2, space="PSUM"))
        ps_t = ctx.enter_context(tc.tile_pool(name="psT", bufs=2, space="PSUM"))
        ps_g = ctx.enter_context(tc.tile_pool(name="psG", bufs=1, space="PSUM"))
        dr_pool = ctx.enter_context(tc.tile_pool(name="dram", bufs=2, space="DRAM"))

        def psg():
            return ps_g.tile([H, 512], F32, name="psg", tag="psg")

        def psg2():
            return ps_g.tile([H, 512], F32, name="psg2", tag="psg2")

        def act_lrelu(dst, src_ps, bias=0.0, accum_out=None):
            if not sim_compat:
                nc.scalar.activation(dst, src_ps, A.Prelu, bias=bias, alpha=0.1,
                                     accum_out=accum_out)
            else:
                shp = [dst.shape[0], int(np.prod(dst.shape[1:]))]
                t1 = sm_pool.tile(shp, F32, name="lr1", tag="lr_t1", bufs=1)
                t2 = sm_pool.tile(shp, F32, name="lr2", tag="lr_t2", bufs=1)
                nb = bias if isinstance(bias, float) else None
                nc.scalar.activation(t1[:], src_ps, A.Relu, bias=bias)
                if nb is None:
                    negb = sm_pool.tile([dst.shape[0], 1], F32, name="lrnb",
                                        tag="lr_nb", bufs=1)
                    nc.vector.tensor_scalar(negb[:], bias, -1.0, None, OP.mult)
                    nc.scalar.activation(t2[:], src_ps, A.Relu, scale=-1.0, bias=negb[:])
                else:
                    nc.scalar.activation(t2[:], src_ps, A.Relu, scale=-1.0, bias=-nb)
                nc.vector.scalar_tensor_tensor(dst, t2[:], -0.1, t1[:],
                                               OP.mult, OP.add, accum_out=accum_out)

        # ---------- preamble ----------
        wp = gl_pool.tile([H, WCOLS], F32R, name="wp", tag="wp")
        bp = gl_pool.tile([H, BCOLS], F32, name="bp", tag="bp")
        bg = gl_pool.tile([G, BGCOLS], F32, name="bg", tag="bg")
        w2t = gl_pool.tile([H, 2 * H], F32, name="w2t", tag="w2t")
        compT = gl_pool.tile([H, G * NC], F32R, name="compT", tag="compT")
        gompT = gl_pool.tile([H, G], F32R, name="gompT", tag="gompT")
        nc.sync.dma_start(out=wp[:], in_=wpack_d[:].bitcast(F32R))
        nc.sync.dma_start(out=bp[:], in_=bpack_d[:])
        nc.sync.dma_start(out=bg[:], in_=bg16_d[:])
        nc.sync.dma_start(out=w2t[:], in_=w2t_d[:])
        nc.sync.dma_start(out=compT[:], in_=compT_d[:].bitcast(F32R))
        nc.sync.dma_start(out=gompT[:], in_=gompT_d[:].bitcast(F32R))

        ident = gl_pool.tile([H, H], F32, name="ident", tag="ident")
        masks.make_identity(nc, ident[:])
        ones_r = gl_pool.tile([H, 2], F32R, name="ones_r", tag="ones_r")
        nc.vector.memset(ones_r[:].bitcast(F32), 1.0)
        identr = ident[:].bitcast(F32R)

        ceT = gl_pool.tile([H, G * NC], F32R, name="ceT", tag="ceT")
        pcT = gl_pool.tile([H, G * NC], F32R, name="pcT", tag="pcT")
        for (dst, wcol, bcol) in ((ceT, W_CAFF, B_CAFF), (pcT, W_PC, B_PC)):
            pscc = ps_big.tile([H, G * NC], F32, name="big", tag="big")
            nc.tensor.matmul(pscc[:, 0:512], wp[:, wcol:wcol + H], compT[:, 0:512],
                             start=True, stop=True)
            nc.tensor.matmul(pscc[:, 512:1024], wp[:, wcol:wcol + H], compT[:, 512:1024],
                             start=True, stop=True)
            act_lrelu(dst[:], pscc[:], bias=bp[:, bcol:bcol + 1])

        # CE_NM [128, 8, 128]: pair-transposed ce (abs graphs 2k, 2k+1 stacked)
        ce_nm = gl_pool.tile([H, 8, H], F32R, name="ce_nm", tag="ce_nm")
        for half in range(2):
            pst = ps_t.tile([H, 512], F32, name="pst", tag="pst")
            for k in range(4):
                pr = half * 4 + k
                nc.tensor.transpose(pst[:, k * 128:(k + 1) * 128],
                                    ceT[:, pr * 128:(pr + 1) * 128].bitcast(F32), ident[:])
            nc.vector.tensor_copy(ce_nm[:, half * 4:(half + 1) * 4, :],
                                  pst[:].rearrange("h (k c) -> h k c", k=4))

        cesum = gl_pool.tile([H, G], F32, name="cesum", tag="cesum")
        nc.vector.tensor_reduce(cesum[:],
                                ceT[:].bitcast(F32).rearrange("h (g c) -> h g c", g=G),
                                AX.X, OP.add)
        peacc = gl_pool.tile([H, G], F32, name="peacc", tag="peacc")
        partials = gl_pool.tile([H, G], F32R, name="partials", tag="partials")

        # ---------- waves ----------
        g0 = 0
        for wv in WAVES:
            gs, ge = g0, g0 + wv
            g0 = ge

            peT = [per_pool.tile([H, NP], F32R, name=f"peT{j}", tag=f"peT{j}") for j in range(wv)]
            pairslab = [per_pool.tile([H, NP], F32R, name=f"pairs{q}", tag=f"pairs{q}")
                        for q in range((wv + 1) // 2)]
            pair = [pairslab[j // 2][(j % 2) * 64:(j % 2) * 64 + 64, :] for j in range(wv)]
            pwe = [per_pool.tile([H, NCHUNK, 65], F32R, name=f"pwe{j}", tag=f"pwe{j}") for j in range(wv)]
            ppe = [per_pool.tile([H, NCHUNK, 256], F32R, name=f"ppe{j}", tag=f"ppe{j}") for j in range(wv)]

            # ----- phase A -----
            for j in range(wv):
                g = gs + j
                protT = st_pool.tile([H, NP], F32R, name="protT", tag="protT")
                nc.sync.dma_start(out=protT[:],
                                  in_=protT_d[:, g * NP:(g + 1) * NP].bitcast(F32R))

                ps_pp = ps_big.tile([H, NP], F32, name="big", tag="big")
                nc.tensor.matmul(ps_pp[:, 0:512], wp[:, W_PP:W_PP + H], protT[:, 0:512],
                                 start=True, stop=True)
                nc.tensor.matmul(ps_pp[:, 512:1024], wp[:, W_PP:W_PP + H],
                                 protT[:, 512:1024], start=True, stop=True)
                ppT = st_pool.tile([H, NP], F32R, name="ppT", tag="ppT")
                act_lrelu(ppT[:], ps_pp[:], bias=bp[:, B_PP:B_PP + 1])

                ps_pe = ps_big.tile([H, NP], F32, name="big", tag="big")
                nc.tensor.matmul(ps_pe[:, 0:512], wp[:, W_PAFF:W_PAFF + H], protT[:, 0:512],
                                 start=True, stop=True)
                nc.tensor.matmul(ps_pe[:, 512:1024], wp[:, W_PAFF:W_PAFF + H],
                                 protT[:, 512:1024], start=True, stop=True)
                act_lrelu(peT[j][:], ps_pe[:], bias=bp[:, B_PAFF:B_PAFF + 1],
                          accum_out=peacc[:, g:g + 1])

                # pairwise = sigmoid(pc @ pp^T) = 0.5 + 0.5*tanh(z/2)
                hb = (j % 2) * 64
                ps_pw = ps_big.tile([H, NP], F32, name="big", tag="big")
                nc.tensor.matmul(ps_pw[0:64, 0:512], pcT[:, g * NC:(g + 1) * NC],
                                 ppT[:, 0:512], start=True, stop=True)
                nc.tensor.matmul(ps_pw[0:64, 512:1024], pcT[:, g * NC:(g + 1) * NC],
                                 ppT[:, 512:1024], start=True, stop=True)
                pw_t = st_pool.tile([H, NP], F32, name="pw_t", tag="pw_t")
                nc.scalar.activation(pw_t[0:64, :], ps_pw[0:64, :], A.Tanh, scale=0.5)
                if hb == 0:
                    nc.vector.tensor_scalar(pair[j], pw_t[0:64, :], 0.5, 0.5,
                                            OP.mult, OP.add)
                else:
                    pair_st = st_pool.tile([64, NP], F32, name="pair_st", tag="pair_st")
                    nc.vector.tensor_scalar(pair_st[:], pw_t[0:64, :], 0.5, 0.5,
                                            OP.mult, OP.add)
                    nc.sync.dma_start(out=pair[j].bitcast(F32), in_=pair_st[:])

                # pairwiseT -> pwe cols 0:64
                for half in range(2):
                    pstp = ps_t.tile([H, 512], F32, name="pst", tag="pst")
                    for k in range(4):
                        ch = half * 4 + k
                        nc.tensor.transpose(pstp[:, k * 128:k * 128 + 64],
                                            pair[j][:, ch * 128:(ch + 1) * 128].bitcast(F32),
                                            ident[hb:hb + 64, hb:hb + 64])
                    nc.vector.tensor_copy(
                        pwe[j][:, half * 4:(half + 1) * 4, 0:64],
                        pstp[:].rearrange("h (k c) -> h k c", k=4)[:, :, 0:64])

                # peT transposes -> ppe cols 128:256 (pe_NM)
                for half in range(2):
                    psq = ps_t.tile([H, 512], F32, name="pst", tag="pst")
                    for k in range(4):
                        ch = half * 4 + k
                        nc.tensor.transpose(psq[:, k * 128:(k + 1) * 128],
                                            peT[j][:, ch * 128:(ch + 1) * 128].bitcast(F32),
                                            ident[:])
                    nc.vector.tensor_copy(
                        ppe[j][:, half * 4:(half + 1) * 4, 128:256],
                        psq[:].rearrange("h (k c) -> h k c", k=4))

            # sf for this wave: lrelu(gomp @ W_saff + b_saff)
            ps_sf = psg()
            nc.tensor.matmul(ps_sf[0:wv, 0:256], gompT[:, gs:ge],
                             wp[:, W_SAFF:W_SAFF + 256], start=True, stop=True)
            sf_pre = sm_pool.tile([wv, H], F32, name="sf_pre", tag="sf_pre", bufs=1)
            nc.vector.tensor_add(sf_pre[:], ps_sf[0:wv, 0:H], bg[0:wv, BG_SAFF:BG_SAFF + H])
            sf_w = sm_pool.tile([wv, H], F32, name="sf_w", tag="sf_w")
            act_lrelu(sf_w[:], sf_pre[:])

            # m0
            mT = sm_pool.tile([H, wv], F32R, name="mT", tag="mT")
            nc.vector.scalar_tensor_tensor(mT[:], cesum[:, gs:ge], 1.0 / (NC * NP),
                                           peacc[:, gs:ge], OP.mult, OP.mult)
            ps_m0 = psg()
            nc.tensor.transpose(ps_m0[0:wv, 0:H], mT[:].bitcast(F32), ident[:])
            m_nm = sm_pool.tile([wv, H], F32, name="m_nm", tag="m_nm")
            nc.vector.tensor_copy(m_nm[:], ps_m0[0:wv, 0:H])

            xcf = sm_pool.tile([wv, H], F32, name="xcf", tag="xcf")
            pfn = sm_pool.tile([wv, H], F32R, name="pfn", tag="pfn")

            # ----- phase B: D iterations -----
            # emit_lead(i): all m-independent leading work of iteration i
            # (c2p/hc0, c_pre_NM, p_pre/hp0 slabs, ppe transposes) so it can
            # be issued ahead of iteration i-1's batched tail and keep the
            # PE fed through the tail's round trips.
            csl = slice(gs * NC, ge * NC)

            def emit_lead(i):
                ps_cp = ps_big.tile([H, wv * NC], F32, name="big", tag="big")
                nc.tensor.matmul(ps_cp[:], wp[:, W_C2P(i):W_C2P(i) + H], ceT[:, csl],
                                 start=True, stop=True)
                cpreT = sm_pool.tile([H, wv * NC], F32, name="cpreT", tag="cpreT")
                nc.scalar.activation(cpreT[:], ps_cp[:], A.Tanh,
                                     bias=bp[:, B_C2P(i):B_C2P(i) + 1])
                ps_h0 = ps_big.tile([H, wv * NC], F32, name="big", tag="big")
                nc.tensor.matmul(ps_h0[:], wp[:, W_HC0(i):W_HC0(i) + H], ceT[:, csl],
                                 start=True, stop=True)
                hc0T = sm_pool.tile([H, wv * NC], F32, name="hc0T", tag="hc0T")
                nc.scalar.activation(hc0T[:], ps_h0[:], A.Tanh,
                                     bias=bp[:, B_HC0(i):B_HC0(i) + 1])

                # c_pre_NM (per-graph 64-col transposes into fixed parity slots)
                cpre_nm = sm_pool.tile([H, 2, H], F32R, name="cpre_nm", tag="cpre_nm")
                psq2 = ps_t.tile([H, 512], F32, name="pst", tag="pst")
                for j in range(wv):
                    nc.tensor.transpose(psq2[0:64, j * 128:(j + 1) * 128],
                                        cpreT[:, j * 64:(j + 1) * 64], ident[:])
                # evens -> partitions 0:64 (DVE), odds -> 64:128 (DMA shifts partitions)
                ne, no = (wv + 1) // 2, wv // 2
                psq2v = psq2[0:64, 0:wv * 128].rearrange("c (j h) -> c j h", j=wv)
                nc.vector.tensor_copy(cpre_nm[0:64, 0:ne, :], psq2v[:, 0::2, :])
                cpre_odd = sm_pool.tile([64, 2, H], F32, name="cpre_odd", tag="cpre_odd", bufs=1)
                nc.vector.tensor_copy(cpre_odd[:, 0:no, :], psq2v[:, 1::2, :])
                nc.sync.dma_start(out=cpre_nm[64:128, 0:no, :].bitcast(F32),
                                  in_=cpre_odd[:, 0:no, :])

                # p_pre/hp0 per graph + p_pre_NM transposes into ppe
                hp0T_l = []
                for j in range(wv):
                    ps_p1 = ps_big.tile([H, NP], F32, name="big", tag="big")
                    nc.tensor.matmul(ps_p1[:, 0:512], wp[:, W_P2C(i):W_P2C(i) + H],
                                     peT[j][:, 0:512], start=True, stop=True)
                    nc.tensor.matmul(ps_p1[:, 512:1024], wp[:, W_P2C(i):W_P2C(i) + H],
                                     peT[j][:, 512:1024], start=True, stop=True)
                    ppreT = st_pool.tile([H, NP], F32, name="ppreT", tag="ppreT")
                    nc.scalar.activation(ppreT[:], ps_p1[:], A.Tanh,
                                         bias=bp[:, B_P2C(i):B_P2C(i) + 1])
                    ps_p2 = ps_big.tile([H, NP], F32, name="big", tag="big")
                    nc.tensor.matmul(ps_p2[:, 0:512], wp[:, W_HP0(i):W_HP0(i) + H],
                                     peT[j][:, 0:512], start=True, stop=True)
                    nc.tensor.matmul(ps_p2[:, 512:1024], wp[:, W_HP0(i):W_HP0(i) + H],
                                     peT[j][:, 512:1024], start=True, stop=True)
                    hp0T = st_pool.tile([H, NP], F32, name="hp0T", tag="hp0T")
                    nc.scalar.activation(hp0T[:], ps_p2[:], A.Tanh,
                                         bias=bp[:, B_HP0(i):B_HP0(i) + 1])
                    hp0T_l.append(hp0T)

                    for half in range(2):
                        psq3 = ps_t.tile([H, 512], F32, name="pst", tag="pst")
                        for k in range(4):
                            ch = half * 4 + k
                            nc.tensor.transpose(psq3[:, k * 128:(k + 1) * 128],
                                                ppreT[:, ch * 128:(ch + 1) * 128], ident[:])
                        nc.vector.tensor_copy(
                            ppe[j][:, half * 4:(half + 1) * 4, 0:128],
                            psq3[:].rearrange("h (k c) -> h k c", k=4))
                return hc0T, cpre_nm, hp0T_l

            lead = emit_lead(0)
            for i in range(D):
                hc0T, cpre_nm, hp0T_l = lead

                # mc1/mp1 batched
                ps_mm = psg()
                nc.tensor.matmul(ps_mm[0:wv, 0:256], mT[:], wp[:, W_MCP(i):W_MCP(i) + 256],
                                 start=True, stop=True)
                mcp_pre = sm_pool.tile([wv, 256], F32, name="mcp_pre", tag="mcp_pre", bufs=1)
                nc.vector.tensor_add(mcp_pre[:], ps_mm[0:wv, 0:256],
                                     bg[0:wv, BG_MCP(i):BG_MCP(i) + 256])
                mcp = sm_pool.tile([wv, 256], F32, name="mcp", tag="mcp")
                nc.scalar.activation(mcp[:], mcp_pre[:], A.Tanh)
                ps_mt = psg()
                nc.tensor.transpose(ps_mt[0:H, 0:wv], mcp[:, 0:H], ident[0:wv, 0:wv])
                mc1T = sm_pool.tile([H, wv], F32, name="mc1T", tag="mc1T")
                nc.vector.tensor_copy(mc1T[:], ps_mt[0:H, 0:wv])
                ps_mt2 = psg()
                nc.tensor.transpose(ps_mt2[0:H, 0:wv], mcp[:, H:256], ident[0:wv, 0:wv])
                mp1T = sm_pool.tile([H, wv], F32, name="mp1T", tag="mp1T")
                nc.vector.tensor_copy(mp1T[:], ps_mt2[0:H, 0:wv])

                wc_w = sm_pool.tile([H, wv], F32, name="wc_w", tag="wc_w")
                nc.vector.tensor_scalar(wc_w[:], mc1T[:],
                                        wp[:, W_HC1(i):W_HC1(i) + 1].bitcast(F32),
                                        None, OP.mult)
                wp_w = sm_pool.tile([H, wv], F32, name="wp_w", tag="wp_w")
                nc.vector.tensor_scalar(wp_w[:], mp1T[:],
                                        wp[:, W_HP1(i):W_HP1(i) + 1].bitcast(F32),
                                        None, OP.mult)

                qcw = sm_pool.tile([H, wv * NC], F32R, name="qcw", tag="qcw")
                esum = sm_pool.tile([1, wv], F32, name="esum", tag="esum")
                pfu = sm_pool.tile([wv, H], F32, name="pfu", tag="pfu")
                pf_stage = sm_pool.tile([H, wv * H], F32, name="pf_stage", tag="pf_stage", bufs=1)

                # ----- per graph heavy chain (pass 1: through e-scatter) -----
                for j in range(wv):
                    g = gs + j
                    hp0T = hp0T_l[j]
                    ps_cp2 = ps_big.tile([H, NP], F32, name="big", tag="big")
                    qb = (j % 2) * 64
                    lhs_cp = cpre_nm[qb:qb + 64, j // 2, :]
                    nc.tensor.matmul(ps_cp2[:, 0:512], lhs_cp, pair[j][:, 0:512],
                                     start=True, stop=True)
                    nc.tensor.matmul(ps_cp2[:, 512:1024], lhs_cp, pair[j][:, 512:1024],
                                     start=True, stop=True)

                    qwT = st_pool.tile([H, NP], F32R, name="qwT", tag="qwT")
                    nc.vector.scalar_tensor_tensor(qwT[:], ps_cp2[:], wp_w[:, j:j + 1],
                                                   hp0T[:], OP.mult, OP.mult)
                    ps_s = ps_big.tile([H, NP], F32, name="big", tag="big")
                    nc.tensor.matmul(ps_s[0:1, 0:512], ones_r[:, 0:1], qwT[:, 0:512],
                                     start=True, stop=True)
                    nc.tensor.matmul(ps_s[0:1, 512:1024], ones_r[:, 0:1], qwT[:, 512:1024],
                                     start=True, stop=True)
                    e_row = st_pool.tile([1, NP], F32, name="e_row", tag="e_row")
                    nc.scalar.activation(e_row[:], ps_s[0:1, :], A.Exp,
                                         accum_out=esum[0:1, j:j + 1])
                    s_dr = dr_pool.tile([NP], F32, name="s_dr", tag="s_dr")
                    nc.sync.dma_start(out=s_dr[:], in_=e_row[:])
                    if bisect:
                        nc.sync.dma_start(out=pwe[j][:, :, 64].bitcast(F32),
                                          in_=s_dr[:].rearrange("(p c) -> p c", c=NCHUNK))
                    else:
                        nc.sync.dma_start(out=pwe[j][:, :, 64].bitcast(F32),
                                          in_=s_dr[:].rearrange("(c p) -> p c", p=128))

                # ----- pass 2: node-contraction per graph -----
                for j in range(wv):
                    g = gs + j
                    ps_cc = ps_g.tile([65, 256], F32, name="psx", tag="psg2")
                    for k in range(NCHUNK):
                        nc.tensor.matmul(ps_cc[:], pwe[j][:, k, :], ppe[j][:, k, :],
                                         start=(k == 0), stop=(k == NCHUNK - 1))
                    p2c = st_pool.tile([64, H], F32, name="p2c", tag="p2c")
                    nc.vector.tensor_copy(p2c[:], ps_cc[0:64, 0:128])
                    ps_tc = ps_t.tile([H, 512], F32, name="pst", tag="pst")
                    nc.tensor.transpose(ps_tc[:, 0:64], p2c[:], ident[0:64, 0:64])
                    nc.vector.scalar_tensor_tensor(qcw[:, j * NC:(j + 1) * NC],
                                                   ps_tc[:, 0:64], wc_w[:, j:j + 1],
                                                   hc0T[:, j * NC:(j + 1) * NC],
                                                   OP.mult, OP.mult)
                    nc.vector.tensor_copy(pf_stage[64:65, j * H:(j + 1) * H],
                                          ps_cc[64:65, 128:256])

                pf_dr = dr_pool.tile([G * H], F32, name="pf_dr", tag="pf_dr")
                nc.sync.dma_start(out=pf_dr[0:wv * H], in_=pf_stage[64:65, :])
                nc.sync.dma_start(out=pfu[:],
                                  in_=pf_dr[0:wv * H].rearrange("(j h) -> j h", j=wv))
                es_dr = dr_pool.tile([G], F32, name="es_dr", tag="es_dr")
                nc.sync.dma_start(out=es_dr[0:wv], in_=esum[:])
                esum_nm = sm_pool.tile([wv, 1], F32, name="esum_nm", tag="esum_nm")
                nc.sync.dma_start(out=esum_nm[:], in_=es_dr[0:wv].rearrange("(j a) -> j a", j=wv))
                esum2 = sm_pool.tile([wv, 1], F32, name="esum2", tag="esum2")
                nc.vector.tensor_scalar(esum2[:], esum_nm[:], 1e-6, None, OP.add)
                rec_p = sm_pool.tile([wv, 1], F32, name="rec_p", tag="rec_p")
                nc.vector.reciprocal(rec_p[:], esum2[:])
                nc.vector.tensor_scalar(pfn[:], pfu[:], rec_p[:], None, OP.mult)

                if i < D - 1:
                    lead = emit_lead(i + 1)

                # ----- batched c softmax + cf + pf + GRU -----
                ps_sc = psg2()
                nc.tensor.matmul(ps_sc[0:1, 0:wv * NC], ones_r[:, 0:1], qcw[:],
                                 start=True, stop=True)
                sc_rowt = sm_pool.tile([1, wv * NC], F32, name="sc_rowt", tag="sc_rowt",
                                       bufs=1)
                nc.scalar.activation(sc_rowt[:], ps_sc[0:1, 0:wv * NC], A.Copy)
                sc_dr = dr_pool.tile([G * NC], F32, name="sc_dr", tag="sc_dr")
                nc.sync.dma_start(out=sc_dr[0:wv * NC], in_=sc_rowt[:])
                sc_nm = sm_pool.tile([wv, NC], F32, name="sc_nm", tag="sc_nm")
                nc.sync.dma_start(out=sc_nm[:],
                                  in_=sc_dr[0:wv * NC].rearrange("(g c) -> g c", g=wv))  # contiguous
                negmax = sm_pool.tile([wv, 1], F32, name="negmax", tag="negmax")
                nc.vector.tensor_reduce(negmax[:], sc_nm[:], AX.X, OP.max, negate=True)
                eac = sm_pool.tile([wv, NC], F32, name="eac", tag="eac")
                sumec = sm_pool.tile([wv, 1], F32, name="sumec", tag="sumec")
                nc.scalar.activation(eac[:], sc_nm[:], A.Exp, bias=negmax[:],
                                     accum_out=sumec[:])
                rec_c = sm_pool.tile([wv, 1], F32, name="rec_c", tag="rec_c")
                nc.vector.reciprocal(rec_c[:], sumec[:])
                ac_nm = sm_pool.tile([wv, NC], F32, name="ac_nm", tag="ac_nm")
                nc.vector.tensor_scalar(ac_nm[:], eac[:], rec_c[:], None, OP.mult)
                # transpose into both parity halves
                ps_at = psg()
                nc.tensor.transpose(ps_at[0:NC, 0:wv], ac_nm[:], ident[0:wv, 0:wv])
                ac_stage = sm_pool.tile([NC, wv], F32, name="ac_stage", tag="ac_stage")
                nc.vector.tensor_copy(ac_stage[:], ps_at[0:NC, 0:wv])
                acT2 = sm_pool.tile([H, wv], F32R, name="acT2", tag="acT2")
                nc.vector.tensor_copy(acT2[0:NC, :], ac_stage[:])
                nc.sync.dma_start(out=acT2[64:128, :].bitcast(F32), in_=ac_stage[:])

                ps_cf = psg2()
                for j in range(wv):
                    g = gs + j
                    hb = (g % 2) * 64
                    nc.tensor.matmul(ps_cf[0:1, j * H:(j + 1) * H],
                                     acT2[hb:hb + 64, j:j + 1],
                                     ce_nm[hb:hb + 64, g // 2, :], start=True, stop=True)
                cf_row = sm_pool.tile([1, wv * H], F32, name="cf_row", tag="cf_row", bufs=1)
                nc.vector.tensor_copy(cf_row[:], ps_cf[0:1, 0:wv * H])
                cf_dr = dr_pool.tile([G * H], F32, name="cf_dr", tag="cf_dr")
                nc.sync.dma_start(out=cf_dr[0:wv * H], in_=cf_row[:])
                nc.sync.dma_start(out=xcf[:],
                                  in_=cf_dr[0:wv * H].rearrange("(j h) -> j h", j=wv))  # contiguous

                if i == D - 1:
                    continue
                xw = sm_pool.tile([wv, H], F32, name="xw", tag="xw")
                nc.vector.tensor_mul(xw[:], xcf[:], pfn[:].bitcast(F32))
                ps_xt = psg()
                nc.tensor.transpose(ps_xt[0:H, 0:wv], xw[:], ident[0:wv, 0:wv])
                xT = sm_pool.tile([H, wv], F32R, name="xT", tag="xT")
                nc.vector.tensor_copy(xT[:], ps_xt[0:H, 0:wv])
                ps_gi = psg()
                nc.tensor.matmul(ps_gi[0:wv, 0:384], xT[:], wp[:, W_IH:W_IH + 384],
                                 start=True, stop=True)
                ps_gh = psg2()
                nc.tensor.matmul(ps_gh[0:wv, 0:384], mT[:], wp[:, W_HH:W_HH + 384],
                                 start=True, stop=True)
                gi = sm_pool.tile([wv, 384], F32, name="gi", tag="gi")
                nc.vector.tensor_add(gi[:], ps_gi[0:wv, 0:384], bg[0:wv, BG_IH:BG_IH + 384])
                gh = sm_pool.tile([wv, 384], F32, name="gh", tag="gh")
                nc.vector.tensor_add(gh[:], ps_gh[0:wv, 0:384], bg[0:wv, BG_HH:BG_HH + 384])
                rz_pre = sm_pool.tile([wv, 256], F32, name="rz_pre", tag="rz_pre", bufs=1)
                nc.vector.tensor_add(rz_pre[:], gi[:, 0:256], gh[:, 0:256])
                rz_t = sm_pool.tile([wv, 256], F32, name="rz_t", tag="rz_t")
                nc.scalar.activation(rz_t[:], rz_pre[:], A.Tanh, scale=0.5)
                rz = sm_pool.tile([wv, 256], F32, name="rz", tag="rz")
                nc.vector.tensor_scalar(rz[:], rz_t[:], 0.5, 0.5, OP.mult, OP.add)
                n_pre = sm_pool.tile([wv, H], F32, name="n_pre", tag="n_pre", bufs=1)
                nc.vector.tensor_mul(n_pre[:], rz[:, 0:H], gh[:, 256:384])
                n_pre2 = sm_pool.tile([wv, H], F32, name="n_pre2", tag="n_pre2", bufs=1)
                nc.vector.tensor_add(n_pre2[:], n_pre[:], gi[:, 256:384])
                n_t = sm_pool.tile([wv, H], F32, name="n_t", tag="n_t")
                nc.scalar.activation(n_t[:], n_pre2[:], A.Tanh)
                dmn = sm_pool.tile([wv, H], F32, name="dmn", tag="dmn", bufs=1)
                nc.vector.tensor_sub(dmn[:], m_nm[:], n_t[:])
                zd = sm_pool.tile([wv, H], F32, name="zd", tag="zd", bufs=1)
                nc.vector.tensor_mul(zd[:], rz[:, H:256], dmn[:])
                m_nm = sm_pool.tile([wv, H], F32, name="m_nm", tag="m_nm")
                nc.vector.tensor_add(m_nm[:], n_t[:], zd[:])
                ps_mT = psg()
                nc.tensor.transpose(ps_mT[0:H, 0:wv], m_nm[:], ident[0:wv, 0:wv])
                mT = sm_pool.tile([H, wv], F32R, name="mT", tag="mT")
                nc.vector.tensor_copy(mT[:], ps_mT[0:H, 0:wv])

            # ----- head -----
            acf = sm_pool.tile([wv, 2 * H], F32R, name="acf", tag="acf")
            nc.vector.tensor_copy(acf[:, 0:H], xcf[:])
            nc.vector.tensor_copy(acf[:, H:2 * H], sf_w[:])
            for j in range(wv):
                g = gs + j
                pfr = st_pool.tile([1, H], F32R, name="pfr", tag="pfr")
                nc.sync.dma_start(out=pfr[:], in_=pfn[j:j + 1, :])
                acfr = st_pool.tile([1, 2 * H], F32R, name="acfr", tag="acfr")
                nc.sync.dma_start(out=acfr[:], in_=acf[j:j + 1, :])
                ps_o = ps_g.tile([H, 256], F32, name="psx", tag="psg2")
                nc.tensor.matmul(ps_o[:], pfr[:], acfr[:],
                                 start=True, stop=True)
                gk = st_pool.tile([H, 2 * H], F32, name="gk", tag="gk")
                act_lrelu(gk[:], ps_o[:])
                gkw = st_pool.tile([H, 2 * H], F32, name="gkw", tag="gkw")
                nc.vector.scalar_tensor_tensor(gkw[:], gk[:], 1.0, w2t[:],
                                               OP.mult, OP.mult,
                                               accum_out=partials[:, g:g + 1])

        # ---------- output ----------
        ps_fin = ps_g.tile([G, 256], F32, name="psx", tag="psg2")
        nc.tensor.matmul(ps_fin[0:G, 0:2], partials[:], ones_r[:], start=True, stop=True)
        ofin = gl_pool.tile([G, 1], F32, name="ofin", tag="ofin")
        nc.vector.tensor_scalar(ofin[:], ps_fin[0:G, 0:1], float(b_out_val), None, OP.add)
        nc.sync.dma_start(out=out_d[:], in_=ofin[:])

    if split:
        _split_waits(nc)
    return nc


def kernel(**inputs) -> np.ndarray:
    f = {k: np.asarray(v) for k, v in inputs.items()}
    f = {k: (v.astype(np.float32) if v.dtype != np.int32 else v) for k, v in f.items()}

    wblocks = [f['W_pc'], f['W_pp'], f['W_caff'], f['W_paff'], f['W_saff']]
    wblocks += [f['W_c2p'][i] for i in range(D)]
    wblocks += [f['W_hc0'][i] for i in range(D)]
    wblocks += [f['W_p2c'][i] for i in range(D)]
    wblocks += [f['W_hp0'][i] for i in range(D)]
    wblocks += [np.concatenate([f['W_mc1'][i], f['W_mp1'][i]], axis=1) for i in range(D)]
    wblocks += [f['W_ih'], f['W_hh']]
    wblocks += [f['W_hc1'][i] for i in range(D)]
    wblocks += [f['W_hp1'][i] for i in range(D)]
    wpack = np.ascontiguousarray(np.concatenate(wblocks, axis=1), dtype=np.float32)
    assert wpack.shape == (H, WCOLS), wpack.shape

    bcols = [f['b_pp'], f['b_paff'], f['b_pc'], f['b_caff']]
    bcols += [f['b_c2p'][i] for i in range(D)]
    bcols += [f['b_hc0'][i] for i in range(D)]
    bcols += [f['b_p2c'][i] for i in range(D)]
    bcols += [f['b_hp0'][i] for i in range(D)]
    bpack = np.stack(bcols, axis=1).astype(np.float32)
    assert bpack.shape == (H, BCOLS)

    bgparts = [np.tile(f['b_saff'][None, :], (G, 1))]
    bgparts += [np.tile(np.concatenate([f['b_mc1'][i], f['b_mp1'][i]])[None, :], (G, 1))
                for i in range(D)]
    bgparts += [np.tile(f['b_ih'][None, :], (G, 1)), np.tile(f['b_hh'][None, :], (G, 1))]
    bg16 = np.concatenate(bgparts, axis=1).astype(np.float32)
    assert bg16.shape == (G, BGCOLS)

    w2t = np.ascontiguousarray(f['W_out'].reshape(2 * H, H).T, dtype=np.float32)
    b_out_val = float(f['b_out'][0])

    key = ('nc', b_out_val)
    if key not in _CACHE:
        _CACHE[key] = _build(b_out_val)
    nc = _CACHE[key]

    comp = f['comp_feature'].reshape(NCORES, G, NC, H)
    prot = f['prot_feature'].reshape(NCORES, G, NP, H)
    gomp = f['gomp_feature'].reshape(NCORES, G, H)
    in_maps = []
    for c in range(NCORES):
        in_maps.append({
            "protT": np.ascontiguousarray(prot[c].transpose(2, 0, 1).reshape(H, G * NP)),
            "compT": np.ascontiguousarray(comp[c].transpose(2, 0, 1).reshape(H, G * NC)),
            "gompT": np.ascontiguousarray(gomp[c].T),
            "wpack": wpack, "bpack": bpack, "bg16": bg16, "w2t": w2t,
        })

    global LAST_EXEC_NS
    try:
        r = run_bass_kernel_spmd(nc, in_maps, list(range(NCORES)), trace=TRACE)
    except ModuleNotFoundError:
        r = run_bass_kernel_spmd(nc, in_maps, list(range(NCORES)))
    if getattr(r, "exec_time_ns", None):
        LAST_EXEC_NS = r.exec_time_ns
    res = r.results
    return np.concatenate([res[c]["out"] for c in range(NCORES)], axis=0)

